# revision 10
# baseline (speedup 1.0000x reference)
"""MiniBatchDiscrimination Trainium2 kernel.

out = concat([x, features], 1) where
  act = (x @ W).reshape(B, K, D)
  l1[b, b2, k] = sum_d |act[b,k,d] - act[b2,k,d]|
  features[b, k] = sum_b2 exp(-l1[b, b2, k])

Sharding: data-parallel over rows b across 8 NeuronCores (64 rows each);
act (full, bf16) is computed replicated on each core.

Per-core device program:
  head: DMA x/W/xb, PE-transpose x, cast bf16, act_T = W^T x^T via PE
        (f on partitions: 6 blocks of 128 + 1 of 32; 512 b columns)
  loop over the 64 owned rows i:
        7x DVE tensor_scalar(sub, abs_max)  -> |act - act[:, own_i]| (bf16, 4x mode)
        7x PE block-diag matmul             -> l1 in PSUM (50 x 512, f32)
        1x ACT Exp(scale=-1) + accum_out    -> features column (50 x 1, f32)
  tail: DMA features (50 x 64) to DRAM

Host side only shards/gathers: slices xb per core, transposes the tiny
(50, 64) feature blocks, and concatenates with x.
"""

import sys

import numpy as np

if "/opt/trn_rl_repo" not in sys.path:
    sys.path.insert(0, "/opt/trn_rl_repo")

import concourse.bass as bass  # noqa: E402
import concourse.tile as tile  # noqa: E402
from concourse import bacc, mybir  # noqa: E402
from concourse.bass_utils import run_bass_kernel_spmd  # noqa: E402
from concourse.masks import make_identity  # noqa: E402

B, F = 512, 512
K, D = 50, 16
KD = K * D  # 800
NCORES = 8
ROWS = B // NCORES  # 64 owned rows per core

# f-axis (k*16+d) blocking over partitions: 6 full blocks + one 32-partition tail
FBLK = [(j * 128, min(128, KD - j * 128)) for j in range((KD + 127) // 128)]

_CACHE: dict = {}


def _emit(nc, tc, x_d, w_d, xb_d, feat_d):
    f32 = mybir.dt.float32
    bf16 = mybir.dt.bfloat16
    Alu = mybir.AluOpType
    Act = mybir.ActivationFunctionType
    nFB = len(FBLK)

    from contextlib import ExitStack

    with ExitStack() as ctx:
        persist = ctx.enter_context(tc.tile_pool(name="persist", bufs=1))
        loads = ctx.enter_context(tc.tile_pool(name="loads", bufs=4))
        ppre = ctx.enter_context(tc.tile_pool(name="ppre", bufs=3, space="PSUM"))
        pl1 = ctx.enter_context(tc.tile_pool(name="pl1", bufs=4, space="PSUM"))
        pabs = ctx.enter_context(tc.tile_pool(name="pabs", bufs=14))
        pex = ctx.enter_context(tc.tile_pool(name="pex", bufs=2))

        # --- constants -------------------------------------------------
        ident = persist.tile([128, 128], f32, tag="ident")
        make_identity(nc, ident[:])

        # Shifted block-diagonal ones for the d-reduction matmuls. bdbig has
        # the 16-partition-group block-diag at columns 48..55; the stationary
        # for f-block j is the 50-column window starting at 48-8j, which puts
        # the ones block at output partitions 8j..8j+nk while keeping the
        # PSUM output base partition at 0 (PE requires base 0/32/64).
        # built via affine_select (engine APs need 32-aligned partition bases,
        # so per-16-partition memsets are not possible):
        #   ones where 0 <= p - 16*(c-48) < 16
        bdf32 = persist.tile([128, 98], f32, tag="bdf32")
        nc.vector.memset(bdf32[:], 1.0)
        nc.gpsimd.affine_select(
            out=bdf32[:], in_=bdf32[:],
            pattern=[[-16, 98]], channel_multiplier=1, base=768,
            compare_op=Alu.is_ge, fill=0.0,
        )
        nc.gpsimd.affine_select(
            out=bdf32[:], in_=bdf32[:],
            pattern=[[16, 98]], channel_multiplier=-1, base=-753,
            compare_op=Alu.is_ge, fill=0.0,
        )
        bdbig = persist.tile([128, 98], bf16, tag="bdbig")
        nc.vector.tensor_copy(bdbig[:], bdf32[:])
        # twos variant: the d-reduction matmul applies the factor 2 in
        # l1 = A2 + S - 2*sum_d min(a, s)
        bd2 = persist.tile([128, 98], bf16, tag="bd2")
        nc.vector.tensor_scalar(
            out=bd2[:], in0=bdf32[:], scalar1=2.0, scalar2=None, op0=Alu.mult
        )

        # --- load & cast W ---------------------------------------------
        w_bf = []
        for i in range(4):
            wt = loads.tile([128, KD], f32, tag="wt")
            nc.sync.dma_start(out=wt[:], in_=w_d[128 * i : 128 * (i + 1), :])
            wb = persist.tile([128, KD], bf16, tag=f"wbf{i}")
            nc.vector.tensor_copy(wb[:], wt[:])
            w_bf.append(wb)

        # --- load x, transpose on PE, cast bf16 ------------------------
        x_sb = []
        for i in range(4):
            xt = loads.tile([128, F], f32, tag=f"xt{i}")
            nc.sync.dma_start(out=xt[:], in_=x_d[128 * i : 128 * (i + 1), :])
            x_sb.append(xt)
        xT_bf = []
        for j in range(4):  # F-blocks (rows of xT)
            pj = ppre.tile([128, B], f32, tag="pp")
            for i in range(4):  # b-blocks (columns of xT)
                nc.tensor.transpose(
                    out=pj[:, 128 * i : 128 * (i + 1)],
                    in_=x_sb[i][:, 128 * j : 128 * (j + 1)],
                    identity=ident[:],
                )
            xb_tile = persist.tile([128, B], bf16, tag=f"xTbf{j}")
            nc.scalar.copy(xb_tile[:], pj[:])
            xT_bf.append(xb_tile)

        # --- load xb (owned rows), transpose, cast ---------------------
        xb_sb = loads.tile([ROWS, F], f32, tag="xb")
        nc.sync.dma_start(out=xb_sb[:], in_=xb_d[:, :])
        xbT_bf = []
        for i in range(4):
            pj = ppre.tile([128, ROWS], f32, tag="pp")
            nc.tensor.transpose(
                out=pj[:],
                in_=xb_sb[:, 128 * i : 128 * (i + 1)],
                identity=ident[0:ROWS, 0:ROWS],
            )
            t = persist.tile([128, ROWS], bf16, tag=f"xbTbf{i}")
            nc.vector.tensor_copy(t[:], pj[:])
            xbT_bf.append(t)

        # --- act_T (full, f on partitions x all 512 b) -----------------
        act_bf = []
        for j, (f0, fp) in enumerate(FBLK):
            pj = ppre.tile([fp, B], f32, tag="pp")
            for i in range(4):
                nc.tensor.matmul(
                    out=pj[:],
                    lhsT=w_bf[i][:, f0 : f0 + fp],
                    rhs=xT_bf[i][:],
                    start=(i == 0),
                    stop=(i == 3),
                )
            ab = persist.tile([fp, B], bf16, tag=f"actbf{j}")
            nc.scalar.copy(ab[:], pj[:])
            act_bf.append(ab)

        # --- act_T own columns (identical values to act_bf's own slice) -
        own_bf = []
        for j, (f0, fp) in enumerate(FBLK):
            pj = ppre.tile([fp, ROWS], f32, tag="pp")
            for i in range(4):
                nc.tensor.matmul(
                    out=pj[:],
                    lhsT=w_bf[i][:, f0 : f0 + fp],
                    rhs=xbT_bf[i][:],
                    start=(i == 0),
                    stop=(i == 3),
                )
            ob = persist.tile([fp, ROWS], bf16, tag=f"ownbf{j}")
            nc.vector.tensor_copy(ob[:], pj[:])
            # f32 copy holding exactly the bf16-rounded values: the DVE scalar
            # operand must be f32, and matching act_bf exactly keeps the
            # self-term |a-a| = 0.
            of = persist.tile([fp, ROWS], f32, tag=f"ownf{j}")
            nc.vector.tensor_copy(of[:], ob[:])
            own_bf.append(of)

        # --- A2[k, b2] = sum_{d in k} act_bf  (i-independent) -----------
        pa2 = ppre.tile([K, B], f32, tag="pp")
        for j, (f0, fp) in enumerate(FBLK):
            nc.tensor.matmul(
                out=pa2[:],
                lhsT=bdbig[0:fp, 48 - 8 * j : 98 - 8 * j],
                rhs=act_bf[j][:],
                start=(j == 0),
                stop=(j == len(FBLK) - 1),
            )
        # f32r copy: operand of the per-i psum-init matmul (-I @ A2). A
        # cross-engine ACT->PSUM init followed by start=False accumulation is
        # nondeterministic on HW, so the init rides the PE accumulation group
        # instead (f32r runs at 1 cyc/row for FD>=256).
        a2r = persist.tile([K, B], mybir.dt.float32r, tag="a2r")
        nc.scalar.copy(a2r[:], pa2[:])

        negI = persist.tile([64, 64], f32, tag="negI")
        make_identity(nc, negI[:])
        negIr = persist.tile([64, 64], mybir.dt.float32r, tag="negIr")
        nc.vector.tensor_scalar(
            out=negIr[:], in0=negI[:], scalar1=-1.0, scalar2=None, op0=Alu.mult
        )

        # --- negS[k, i] = -sum_{d in k} own col (f32 matmul) ------------
        ps = ppre.tile([K, ROWS], f32, tag="pp")
        for j, (f0, fp) in enumerate(FBLK):
            nc.tensor.matmul(
                out=ps[:],
                lhsT=bdf32[0:fp, 48 - 8 * j : 98 - 8 * j],
                rhs=own_bf[j][:],
                start=(j == 0),
                stop=(j == len(FBLK) - 1),
            )
        negS = persist.tile([K, ROWS], f32, tag="negS")
        nc.scalar.activation(negS[:], ps[:], Act.Copy, bias=0.0, scale=-1.0)

        # --- features accumulator --------------------------------------
        feat = persist.tile([K, ROWS], f32, tag="feat")

        # --- main loop over owned rows ---------------------------------
        # l1 = A2 + S - 2*sum_d min(act, own_i)  (|a-s| = a + s - 2 min(a,s))
        # psum P := -A2 + 2*sum_d min  -> features col = sum_b2 exp(P - S)
        for i in range(ROWS):
            l1 = pl1.tile([K, B], f32, tag="l1")
            nc.tensor.matmul(
                out=l1[:],
                lhsT=negIr[0:K, 0:K],
                rhs=a2r[:],
                start=True,
                stop=False,
            )
            for j, (f0, fp) in enumerate(FBLK):
                ab = pabs.tile([fp, B], bf16, tag="ab")
                nc.vector.tensor_scalar(
                    out=ab[:],
                    in0=act_bf[j][:],
                    scalar1=own_bf[j][:, i : i + 1],
                    scalar2=None,
                    op0=Alu.min,
                )
                nc.tensor.matmul(
                    out=l1[:],
                    lhsT=bd2[0:fp, 48 - 8 * j : 98 - 8 * j],
                    rhs=ab[:],
                    start=False,
                    stop=(j == len(FBLK) - 1),
                )
            scratch = pex.tile([K, B], bf16, tag="ex")
            nc.scalar.activation(
                out=scratch[:],
                in_=l1[:],
                func=Act.Exp,
                bias=negS[:, i : i + 1],
                scale=1.0,
                accum_out=feat[:, i : i + 1],
            )

        nc.sync.dma_start(out=feat_d[:, :], in_=feat[:])


def _build():
    if "nc" in _CACHE:
        return _CACHE["nc"]
    nc = bacc.Bacc("TRN2", target_bir_lowering=False, debug=False, num_devices=NCORES)
    x_d = nc.dram_tensor("x", [B, F], mybir.dt.float32, kind="ExternalInput").ap()
    w_d = nc.dram_tensor("w", [F, KD], mybir.dt.float32, kind="ExternalInput").ap()
    xb_d = nc.dram_tensor("xb", [ROWS, F], mybir.dt.float32, kind="ExternalInput").ap()
    feat_d = nc.dram_tensor(
        "feat", [K, ROWS], mybir.dt.float32, kind="ExternalOutput"
    ).ap()
    with tile.TileContext(nc) as tc:
        _emit(nc, tc, x_d, w_d, xb_d, feat_d)
    nc.compile()
    _CACHE["nc"] = nc
    return nc


def kernel(x, W):
    x = np.ascontiguousarray(np.asarray(x, dtype=np.float32))
    W = np.ascontiguousarray(np.asarray(W, dtype=np.float32))
    assert x.shape == (B, F) and W.shape == (F, KD)

    nc = _build()
    in_maps = [
        {"x": x, "w": W, "xb": np.ascontiguousarray(x[c * ROWS : (c + 1) * ROWS, :])}
        for c in range(NCORES)
    ]
    res = run_bass_kernel_spmd(nc, in_maps, core_ids=list(range(NCORES)))

    out = np.empty((B, F + K), dtype=np.float32)
    out[:, :F] = x
    for c in range(NCORES):
        out[c * ROWS : (c + 1) * ROWS, F:] = res.results[c]["feat"].T
    return out


# revision 13
# speedup vs baseline: 1.1042x; 1.1042x over previous
"""MiniBatchDiscrimination Trainium2 kernel.

out = concat([x, features], 1) where
  act = (x @ W).reshape(B, K, D)
  l1[b, b2, k] = sum_d |act[b,k,d] - act[b2,k,d]|
  features[b, k] = sum_b2 exp(-l1[b, b2, k])

Sharding: data-parallel over rows b across 8 NeuronCores (64 rows each);
act (full, bf16) is computed replicated on each core.

Per-core device program:
  head: DMA x/W/xb, PE-transpose x, cast bf16, act_T = W^T x^T via PE
        (f on partitions: 6 blocks of 128 + 1 of 32; 512 b columns)
  loop over the 64 owned rows i:
        7x DVE tensor_scalar(sub, abs_max)  -> |act - act[:, own_i]| (bf16, 4x mode)
        7x PE block-diag matmul             -> l1 in PSUM (50 x 512, f32)
        1x ACT Exp(scale=-1) + accum_out    -> features column (50 x 1, f32)
  tail: DMA features (50 x 64) to DRAM

Host side only shards/gathers: slices xb per core, transposes the tiny
(50, 64) feature blocks, and concatenates with x.
"""

import sys

import numpy as np

if "/opt/trn_rl_repo" not in sys.path:
    sys.path.insert(0, "/opt/trn_rl_repo")

import concourse.bass as bass  # noqa: E402
import concourse.tile as tile  # noqa: E402
from concourse import bacc, mybir  # noqa: E402
from concourse.bass_utils import run_bass_kernel_spmd  # noqa: E402
from concourse.masks import make_identity  # noqa: E402

B, F = 512, 512
K, D = 50, 16
KD = K * D  # 800
NCORES = 8
ROWS = B // NCORES  # 64 owned rows per core

# f-axis (k*16+d) blocking over partitions: 6 full blocks + one 32-partition tail
FBLK = [(j * 128, min(128, KD - j * 128)) for j in range((KD + 127) // 128)]

_CACHE: dict = {}


def _emit(nc, tc, x_d, w_d, xb_d, feat_d):
    f32 = mybir.dt.float32
    bf16 = mybir.dt.bfloat16
    Alu = mybir.AluOpType
    Act = mybir.ActivationFunctionType
    nFB = len(FBLK)

    from contextlib import ExitStack

    with ExitStack() as ctx:
        persist = ctx.enter_context(tc.tile_pool(name="persist", bufs=1))
        loads = ctx.enter_context(tc.tile_pool(name="loads", bufs=4))
        ppre = ctx.enter_context(tc.tile_pool(name="ppre", bufs=3, space="PSUM"))
        pl1 = ctx.enter_context(tc.tile_pool(name="pl1", bufs=4, space="PSUM"))
        pabs = ctx.enter_context(tc.tile_pool(name="pabs", bufs=14))
        pex = ctx.enter_context(tc.tile_pool(name="pex", bufs=2))

        # --- constants -------------------------------------------------
        ident = persist.tile([128, 128], f32, tag="ident")
        make_identity(nc, ident[:])

        # Shifted block-diagonal ones for the d-reduction matmuls. bdbig has
        # the 16-partition-group block-diag at columns 48..55; the stationary
        # for f-block j is the 50-column window starting at 48-8j, which puts
        # the ones block at output partitions 8j..8j+nk while keeping the
        # PSUM output base partition at 0 (PE requires base 0/32/64).
        # built via affine_select (engine APs need 32-aligned partition bases,
        # so per-16-partition memsets are not possible):
        #   ones where 0 <= p - 16*(c-48) < 16
        bdf32 = persist.tile([128, 98], f32, tag="bdf32")
        nc.vector.memset(bdf32[:], 1.0)
        nc.gpsimd.affine_select(
            out=bdf32[:], in_=bdf32[:],
            pattern=[[-16, 98]], channel_multiplier=1, base=768,
            compare_op=Alu.is_ge, fill=0.0,
        )
        nc.gpsimd.affine_select(
            out=bdf32[:], in_=bdf32[:],
            pattern=[[16, 98]], channel_multiplier=-1, base=-753,
            compare_op=Alu.is_ge, fill=0.0,
        )
        bdbig = persist.tile([128, 98], bf16, tag="bdbig")
        nc.vector.tensor_copy(bdbig[:], bdf32[:])
        # twos variant: the d-reduction matmul applies the factor 2 in
        # l1 = A2 + S - 2*sum_d min(a, s)
        bd2 = persist.tile([128, 98], bf16, tag="bd2")
        nc.vector.tensor_scalar(
            out=bd2[:], in0=bdf32[:], scalar1=2.0, scalar2=None, op0=Alu.mult
        )

        # --- load & cast W ---------------------------------------------
        w_bf = []
        for i in range(4):
            wt = loads.tile([128, KD], f32, tag="wt")
            nc.sync.dma_start(out=wt[:], in_=w_d[128 * i : 128 * (i + 1), :])
            wb = persist.tile([128, KD], bf16, tag=f"wbf{i}")
            nc.vector.tensor_copy(wb[:], wt[:])
            w_bf.append(wb)

        # --- load x, transpose on PE, cast bf16 ------------------------
        x_sb = []
        for i in range(4):
            xt = loads.tile([128, F], f32, tag=f"xt{i}")
            nc.sync.dma_start(out=xt[:], in_=x_d[128 * i : 128 * (i + 1), :])
            x_sb.append(xt)
        xT_bf = []
        for j in range(4):  # F-blocks (rows of xT)
            pj = ppre.tile([128, B], f32, tag="pp")
            for i in range(4):  # b-blocks (columns of xT)
                nc.tensor.transpose(
                    out=pj[:, 128 * i : 128 * (i + 1)],
                    in_=x_sb[i][:, 128 * j : 128 * (j + 1)],
                    identity=ident[:],
                )
            xb_tile = persist.tile([128, B], bf16, tag=f"xTbf{j}")
            nc.scalar.copy(xb_tile[:], pj[:])
            xT_bf.append(xb_tile)

        # --- load xb (owned rows), transpose, cast ---------------------
        xb_sb = loads.tile([ROWS, F], f32, tag="xb")
        nc.sync.dma_start(out=xb_sb[:], in_=xb_d[:, :])
        xbT_bf = []
        for i in range(4):
            pj = ppre.tile([128, ROWS], f32, tag="pp")
            nc.tensor.transpose(
                out=pj[:],
                in_=xb_sb[:, 128 * i : 128 * (i + 1)],
                identity=ident[0:ROWS, 0:ROWS],
            )
            t = persist.tile([128, ROWS], bf16, tag=f"xbTbf{i}")
            nc.vector.tensor_copy(t[:], pj[:])
            xbT_bf.append(t)

        # --- act_T (full, f on partitions x all 512 b) -----------------
        act_bf = []
        for j, (f0, fp) in enumerate(FBLK):
            pj = ppre.tile([fp, B], f32, tag="pp")
            for i in range(4):
                nc.tensor.matmul(
                    out=pj[:],
                    lhsT=w_bf[i][:, f0 : f0 + fp],
                    rhs=xT_bf[i][:],
                    start=(i == 0),
                    stop=(i == 3),
                )
            ab = persist.tile([fp, B], bf16, tag=f"actbf{j}")
            nc.scalar.copy(ab[:], pj[:])
            act_bf.append(ab)

        # --- act_T own columns (identical values to act_bf's own slice) -
        own_bf = []
        for j, (f0, fp) in enumerate(FBLK):
            pj = ppre.tile([fp, ROWS], f32, tag="pp")
            for i in range(4):
                nc.tensor.matmul(
                    out=pj[:],
                    lhsT=w_bf[i][:, f0 : f0 + fp],
                    rhs=xbT_bf[i][:],
                    start=(i == 0),
                    stop=(i == 3),
                )
            ob = persist.tile([fp, ROWS], bf16, tag=f"ownbf{j}")
            nc.vector.tensor_copy(ob[:], pj[:])
            # f32 copy holding exactly the bf16-rounded values: the DVE scalar
            # operand must be f32, and matching act_bf exactly keeps the
            # self-term |a-a| = 0.
            of = persist.tile([fp, ROWS], f32, tag=f"ownf{j}")
            nc.vector.tensor_copy(of[:], ob[:])
            own_bf.append(of)

        # --- A2[k, b2] = sum_{d in k} act_bf  (i-independent) -----------
        pa2 = ppre.tile([K, B], f32, tag="pp")
        for j, (f0, fp) in enumerate(FBLK):
            nc.tensor.matmul(
                out=pa2[:],
                lhsT=bdbig[0:fp, 48 - 8 * j : 98 - 8 * j],
                rhs=act_bf[j][:],
                start=(j == 0),
                stop=(j == len(FBLK) - 1),
            )
        # The -A2 psum-init rides the PE accumulation group (a cross-engine
        # ACT->PSUM init followed by start=False accumulation is
        # nondeterministic on HW). It is merged with the 32-partition leftover
        # f-block (kernels 48/49) into one f32r matmul: moving tile m6
        # (96, 512) holds A2 in rows 0-49 and the per-i leftover min values in
        # rows 64-95; stationary s6 has diag(-1) on rows 0-49 and 2.0 blocks
        # mapping rows 64-95 to kernels 48/49. f32r runs 1 cyc/row at FD>=256.
        f32r = mybir.dt.float32r
        s6f = persist.tile([96, K], f32, tag="s6f")
        nc.vector.memset(s6f[:], 0.0)
        nc.gpsimd.affine_select(
            out=s6f[:], in_=s6f[:],
            pattern=[[-1, K]], channel_multiplier=1, base=0,
            compare_op=Alu.not_equal, fill=-1.0,
        )
        aux = persist.tile([96, K], f32, tag="aux")
        nc.vector.memset(aux[:], 2.0)
        # keep where 0 <= (p - 64) - 16*(c - 48) < 16 and p >= 64, else 0
        nc.gpsimd.affine_select(
            out=aux[:], in_=aux[:],
            pattern=[[-16, K]], channel_multiplier=1, base=704,
            compare_op=Alu.is_ge, fill=0.0,
        )
        nc.gpsimd.affine_select(
            out=aux[:], in_=aux[:],
            pattern=[[16, K]], channel_multiplier=-1, base=-689,
            compare_op=Alu.is_ge, fill=0.0,
        )
        nc.gpsimd.affine_select(
            out=aux[:], in_=aux[:],
            pattern=[[0, K]], channel_multiplier=1, base=-64,
            compare_op=Alu.is_ge, fill=0.0,
        )
        nc.vector.tensor_tensor(s6f[:], s6f[:], aux[:], op=Alu.add)
        s6r = persist.tile([96, K], f32r, tag="s6r")
        nc.vector.tensor_copy(s6r[:], s6f[:])

        # triple-buffered combined moving tiles; A2 rows written once.
        # rows 50-63 are multiplied by zero stationary weights but must not
        # hold uninitialized bits (0 * Inf = NaN in the PE), and memset can't
        # produce f32r -- zero them via a DVE copy from an f32 zeros tile.
        zf = loads.tile([32, B], f32, tag="zf")
        nc.vector.memset(zf[:], 0.0)
        m6 = []
        for b in range(3):
            t = persist.tile([96, B], f32r, tag=f"m6_{b}")
            nc.vector.tensor_copy(t[32:64, :], zf[:])
            nc.vector.tensor_copy(t[0:K, :], pa2[:])
            m6.append(t)

        # --- negS[k, i] = -sum_{d in k} own col (f32 matmul) ------------
        ps = ppre.tile([K, ROWS], f32, tag="pp")
        for j, (f0, fp) in enumerate(FBLK):
            nc.tensor.matmul(
                out=ps[:],
                lhsT=bdf32[0:fp, 48 - 8 * j : 98 - 8 * j],
                rhs=own_bf[j][:],
                start=(j == 0),
                stop=(j == len(FBLK) - 1),
            )
        negS = persist.tile([K, ROWS], f32, tag="negS")
        nc.scalar.activation(negS[:], ps[:], Act.Copy, bias=0.0, scale=-1.0)

        # --- features accumulator --------------------------------------
        feat = persist.tile([K, ROWS], f32, tag="feat")

        # --- main loop over owned rows ---------------------------------
        # l1 = A2 + S - 2*sum_d min(act, own_i)  (|a-s| = a + s - 2 min(a,s))
        # psum P := -A2 + 2*sum_d min  -> features col = sum_b2 exp(P - S)
        for i in range(ROWS):
            l1 = pl1.tile([K, B], f32, tag="l1")
            mb_ = m6[i % 3]
            nc.vector.tensor_scalar(
                out=mb_[64:96, :],
                in0=act_bf[6][:],
                scalar1=own_bf[6][:, i : i + 1],
                scalar2=None,
                op0=Alu.min,
            )
            nc.tensor.matmul(
                out=l1[:], lhsT=s6r[:], rhs=mb_[:], start=True, stop=False
            )
            for j in range(6):
                ab = pabs.tile([128, B], bf16, tag="ab")
                nc.vector.tensor_scalar(
                    out=ab[:],
                    in0=act_bf[j][:],
                    scalar1=own_bf[j][:, i : i + 1],
                    scalar2=None,
                    op0=Alu.min,
                )
                nc.tensor.matmul(
                    out=l1[:],
                    lhsT=bd2[0:128, 48 - 8 * j : 98 - 8 * j],
                    rhs=ab[:],
                    start=False,
                    stop=(j == 5),
                )
            scratch = pex.tile([K, B], bf16, tag="ex")
            nc.scalar.activation(
                out=scratch[:],
                in_=l1[:],
                func=Act.Exp,
                bias=negS[:, i : i + 1],
                scale=1.0,
                accum_out=feat[:, i : i + 1],
            )

        nc.sync.dma_start(out=feat_d[:, :], in_=feat[:])


def _build():
    if "nc" in _CACHE:
        return _CACHE["nc"]
    nc = bacc.Bacc("TRN2", target_bir_lowering=False, debug=False, num_devices=NCORES)
    x_d = nc.dram_tensor("x", [B, F], mybir.dt.float32, kind="ExternalInput").ap()
    w_d = nc.dram_tensor("w", [F, KD], mybir.dt.float32, kind="ExternalInput").ap()
    xb_d = nc.dram_tensor("xb", [ROWS, F], mybir.dt.float32, kind="ExternalInput").ap()
    feat_d = nc.dram_tensor(
        "feat", [K, ROWS], mybir.dt.float32, kind="ExternalOutput"
    ).ap()
    with tile.TileContext(nc) as tc:
        _emit(nc, tc, x_d, w_d, xb_d, feat_d)
    nc.compile()
    _CACHE["nc"] = nc
    return nc


def kernel(x, W):
    x = np.ascontiguousarray(np.asarray(x, dtype=np.float32))
    W = np.ascontiguousarray(np.asarray(W, dtype=np.float32))
    assert x.shape == (B, F) and W.shape == (F, KD)

    nc = _build()
    in_maps = [
        {"x": x, "w": W, "xb": np.ascontiguousarray(x[c * ROWS : (c + 1) * ROWS, :])}
        for c in range(NCORES)
    ]
    res = run_bass_kernel_spmd(nc, in_maps, core_ids=list(range(NCORES)))

    out = np.empty((B, F + K), dtype=np.float32)
    out[:, :F] = x
    for c in range(NCORES):
        out[c * ROWS : (c + 1) * ROWS, F:] = res.results[c]["feat"].T
    return out


# revision 14
# speedup vs baseline: 1.1053x; 1.0010x over previous
"""MiniBatchDiscrimination Trainium2 kernel.

out = concat([x, features], 1) where
  act = (x @ W).reshape(B, K, D)
  l1[b, b2, k] = sum_d |act[b,k,d] - act[b2,k,d]|
  features[b, k] = sum_b2 exp(-l1[b, b2, k])

Sharding: data-parallel over rows b across 8 NeuronCores (64 rows each);
act (full, bf16) is computed replicated on each core.

Per-core device program:
  head: DMA x/W/xb, PE-transpose x, cast bf16, act_T = W^T x^T via PE
        (f on partitions: 6 blocks of 128 + 1 of 32; 512 b columns)
  loop over the 64 owned rows i:
        7x DVE tensor_scalar(sub, abs_max)  -> |act - act[:, own_i]| (bf16, 4x mode)
        7x PE block-diag matmul             -> l1 in PSUM (50 x 512, f32)
        1x ACT Exp(scale=-1) + accum_out    -> features column (50 x 1, f32)
  tail: DMA features (50 x 64) to DRAM

Host side only shards/gathers: slices xb per core, transposes the tiny
(50, 64) feature blocks, and concatenates with x.
"""

import sys

import numpy as np

if "/opt/trn_rl_repo" not in sys.path:
    sys.path.insert(0, "/opt/trn_rl_repo")

import concourse.bass as bass  # noqa: E402
import concourse.tile as tile  # noqa: E402
from concourse import bacc, mybir  # noqa: E402
from concourse.bass_utils import run_bass_kernel_spmd  # noqa: E402
from concourse.masks import make_identity  # noqa: E402

B, F = 512, 512
K, D = 50, 16
KD = K * D  # 800
NCORES = 8
ROWS = B // NCORES  # 64 owned rows per core

# f-axis (k*16+d) blocking over partitions: 6 full blocks + one 32-partition tail
FBLK = [(j * 128, min(128, KD - j * 128)) for j in range((KD + 127) // 128)]

_CACHE: dict = {}


def _emit(nc, tc, x_d, w_d, xb_d, feat_d):
    f32 = mybir.dt.float32
    bf16 = mybir.dt.bfloat16
    Alu = mybir.AluOpType
    Act = mybir.ActivationFunctionType
    nFB = len(FBLK)

    from contextlib import ExitStack

    with ExitStack() as ctx:
        persist = ctx.enter_context(tc.tile_pool(name="persist", bufs=1))
        loads = ctx.enter_context(tc.tile_pool(name="loads", bufs=4))
        ppre = ctx.enter_context(tc.tile_pool(name="ppre", bufs=3, space="PSUM"))
        pl1 = ctx.enter_context(tc.tile_pool(name="pl1", bufs=4, space="PSUM"))
        pabs = ctx.enter_context(tc.tile_pool(name="pabs", bufs=14))
        pex = ctx.enter_context(tc.tile_pool(name="pex", bufs=2))

        # --- constants -------------------------------------------------
        ident = persist.tile([128, 128], f32, tag="ident")
        make_identity(nc, ident[:])

        # Shifted block-diagonal ones for the d-reduction matmuls. bdbig has
        # the 16-partition-group block-diag at columns 48..55; the stationary
        # for f-block j is the 50-column window starting at 48-8j, which puts
        # the ones block at output partitions 8j..8j+nk while keeping the
        # PSUM output base partition at 0 (PE requires base 0/32/64).
        # built via affine_select (engine APs need 32-aligned partition bases,
        # so per-16-partition memsets are not possible):
        #   ones where 0 <= p - 16*(c-48) < 16
        bdf32 = persist.tile([128, 98], f32, tag="bdf32")
        nc.vector.memset(bdf32[:], 1.0)
        nc.gpsimd.affine_select(
            out=bdf32[:], in_=bdf32[:],
            pattern=[[-16, 98]], channel_multiplier=1, base=768,
            compare_op=Alu.is_ge, fill=0.0,
        )
        nc.gpsimd.affine_select(
            out=bdf32[:], in_=bdf32[:],
            pattern=[[16, 98]], channel_multiplier=-1, base=-753,
            compare_op=Alu.is_ge, fill=0.0,
        )
        bdbig = persist.tile([128, 98], bf16, tag="bdbig")
        nc.vector.tensor_copy(bdbig[:], bdf32[:])
        # twos variant: the d-reduction matmul applies the factor 2 in
        # l1 = A2 + S - 2*sum_d min(a, s)
        bd2 = persist.tile([128, 98], bf16, tag="bd2")
        nc.vector.tensor_scalar(
            out=bd2[:], in0=bdf32[:], scalar1=2.0, scalar2=None, op0=Alu.mult
        )

        # --- load & cast W ---------------------------------------------
        w_bf = []
        for i in range(4):
            wt = loads.tile([128, KD], f32, tag="wt")
            nc.sync.dma_start(out=wt[:], in_=w_d[128 * i : 128 * (i + 1), :])
            wb = persist.tile([128, KD], bf16, tag=f"wbf{i}")
            nc.vector.tensor_copy(wb[:], wt[:])
            w_bf.append(wb)

        # --- load x, transpose on PE, cast bf16 ------------------------
        x_sb = []
        for i in range(4):
            xt = loads.tile([128, F], f32, tag=f"xt{i}")
            nc.sync.dma_start(out=xt[:], in_=x_d[128 * i : 128 * (i + 1), :])
            x_sb.append(xt)
        xT_bf = []
        for j in range(4):  # F-blocks (rows of xT)
            pj = ppre.tile([128, B], f32, tag="pp")
            for i in range(4):  # b-blocks (columns of xT)
                nc.tensor.transpose(
                    out=pj[:, 128 * i : 128 * (i + 1)],
                    in_=x_sb[i][:, 128 * j : 128 * (j + 1)],
                    identity=ident[:],
                )
            xb_tile = persist.tile([128, B], bf16, tag=f"xTbf{j}")
            nc.scalar.copy(xb_tile[:], pj[:])
            xT_bf.append(xb_tile)

        # --- load xb (owned rows), transpose, cast ---------------------
        xb_sb = loads.tile([ROWS, F], f32, tag="xb")
        nc.sync.dma_start(out=xb_sb[:], in_=xb_d[:, :])
        xbT_bf = []
        for i in range(4):
            pj = ppre.tile([128, ROWS], f32, tag="pp")
            nc.tensor.transpose(
                out=pj[:],
                in_=xb_sb[:, 128 * i : 128 * (i + 1)],
                identity=ident[0:ROWS, 0:ROWS],
            )
            t = persist.tile([128, ROWS], bf16, tag=f"xbTbf{i}")
            nc.vector.tensor_copy(t[:], pj[:])
            xbT_bf.append(t)

        # --- act_T (full, f on partitions x all 512 b) -----------------
        act_bf = []
        for j, (f0, fp) in enumerate(FBLK):
            pj = ppre.tile([fp, B], f32, tag="pp")
            for i in range(4):
                nc.tensor.matmul(
                    out=pj[:],
                    lhsT=w_bf[i][:, f0 : f0 + fp],
                    rhs=xT_bf[i][:],
                    start=(i == 0),
                    stop=(i == 3),
                )
            ab = persist.tile([fp, B], bf16, tag=f"actbf{j}")
            nc.scalar.copy(ab[:], pj[:])
            act_bf.append(ab)

        # --- act_T own columns (identical values to act_bf's own slice) -
        own_bf = []
        for j, (f0, fp) in enumerate(FBLK):
            pj = ppre.tile([fp, ROWS], f32, tag="pp")
            for i in range(4):
                nc.tensor.matmul(
                    out=pj[:],
                    lhsT=w_bf[i][:, f0 : f0 + fp],
                    rhs=xbT_bf[i][:],
                    start=(i == 0),
                    stop=(i == 3),
                )
            ob = persist.tile([fp, ROWS], bf16, tag=f"ownbf{j}")
            nc.vector.tensor_copy(ob[:], pj[:])
            # f32 copy holding exactly the bf16-rounded values: the DVE scalar
            # operand must be f32, and matching act_bf exactly keeps the
            # self-term |a-a| = 0.
            of = persist.tile([fp, ROWS], f32, tag=f"ownf{j}")
            nc.vector.tensor_copy(of[:], ob[:])
            own_bf.append(of)

        # --- A2[k, b2] = sum_{d in k} act_bf  (i-independent) -----------
        pa2 = ppre.tile([K, B], f32, tag="pp")
        for j, (f0, fp) in enumerate(FBLK):
            nc.tensor.matmul(
                out=pa2[:],
                lhsT=bdbig[0:fp, 48 - 8 * j : 98 - 8 * j],
                rhs=act_bf[j][:],
                start=(j == 0),
                stop=(j == len(FBLK) - 1),
            )
        # The -A2 psum-init rides the PE accumulation group (a cross-engine
        # ACT->PSUM init followed by start=False accumulation is
        # nondeterministic on HW). It is merged with the 32-partition leftover
        # f-block (kernels 48/49) into one f32r matmul: moving tile m6
        # (96, 512) holds A2 in rows 0-49 and the per-i leftover min values in
        # rows 64-95; stationary s6 has diag(-1) on rows 0-49 and 2.0 blocks
        # mapping rows 64-95 to kernels 48/49. f32r runs 1 cyc/row at FD>=256.
        f32r = mybir.dt.float32r
        s6f = persist.tile([96, K], f32, tag="s6f")
        nc.vector.memset(s6f[:], 0.0)
        nc.gpsimd.affine_select(
            out=s6f[:], in_=s6f[:],
            pattern=[[-1, K]], channel_multiplier=1, base=0,
            compare_op=Alu.not_equal, fill=-1.0,
        )
        aux = persist.tile([96, K], f32, tag="aux")
        nc.vector.memset(aux[:], 2.0)
        # keep where 0 <= (p - 64) - 16*(c - 48) < 16 and p >= 64, else 0
        nc.gpsimd.affine_select(
            out=aux[:], in_=aux[:],
            pattern=[[-16, K]], channel_multiplier=1, base=704,
            compare_op=Alu.is_ge, fill=0.0,
        )
        nc.gpsimd.affine_select(
            out=aux[:], in_=aux[:],
            pattern=[[16, K]], channel_multiplier=-1, base=-689,
            compare_op=Alu.is_ge, fill=0.0,
        )
        nc.gpsimd.affine_select(
            out=aux[:], in_=aux[:],
            pattern=[[0, K]], channel_multiplier=1, base=-64,
            compare_op=Alu.is_ge, fill=0.0,
        )
        nc.vector.tensor_tensor(s6f[:], s6f[:], aux[:], op=Alu.add)
        s6r = persist.tile([96, K], f32r, tag="s6r")
        nc.vector.tensor_copy(s6r[:], s6f[:])

        # triple-buffered combined moving tiles; A2 rows written once.
        # rows 50-63 are multiplied by zero stationary weights but must not
        # hold uninitialized bits (0 * Inf = NaN in the PE), and memset can't
        # produce f32r -- zero them via a DVE copy from an f32 zeros tile.
        zf = loads.tile([32, B], f32, tag="zf")
        nc.vector.memset(zf[:], 0.0)
        m6 = []
        for b in range(3):
            t = persist.tile([96, B], f32r, tag=f"m6_{b}")
            nc.vector.tensor_copy(t[32:64, :], zf[:])
            nc.vector.tensor_copy(t[0:K, :], pa2[:])
            m6.append(t)

        # --- negS[k, i] = -sum_{d in k} own col (f32 matmul) ------------
        ps = ppre.tile([K, ROWS], f32, tag="pp")
        for j, (f0, fp) in enumerate(FBLK):
            nc.tensor.matmul(
                out=ps[:],
                lhsT=bdf32[0:fp, 48 - 8 * j : 98 - 8 * j],
                rhs=own_bf[j][:],
                start=(j == 0),
                stop=(j == len(FBLK) - 1),
            )
        negS = persist.tile([K, ROWS], f32, tag="negS")
        nc.scalar.activation(negS[:], ps[:], Act.Copy, bias=0.0, scale=-1.0)

        # --- features accumulator --------------------------------------
        feat = persist.tile([K, ROWS], f32, tag="feat")

        # --- main loop over owned rows ---------------------------------
        # l1 = A2 + S - 2*sum_d min(act, own_i)  (|a-s| = a + s - 2 min(a,s))
        # psum P := -A2 + 2*sum_d min  -> features col = sum_b2 exp(P - S)
        for i in range(ROWS):
            l1 = pl1.tile([K, B], f32, tag="l1")
            # j=0 starts the group (its inputs are ready earliest in the
            # head); the A2-dependent merged leftover matmul goes last.
            for j in range(6):
                ab = pabs.tile([128, B], bf16, tag="ab")
                nc.vector.tensor_scalar(
                    out=ab[:],
                    in0=act_bf[j][:],
                    scalar1=own_bf[j][:, i : i + 1],
                    scalar2=None,
                    op0=Alu.min,
                )
                nc.tensor.matmul(
                    out=l1[:],
                    lhsT=bd2[0:128, 48 - 8 * j : 98 - 8 * j],
                    rhs=ab[:],
                    start=(j == 0),
                    stop=False,
                )
            mb_ = m6[i % 3]
            nc.vector.tensor_scalar(
                out=mb_[64:96, :],
                in0=act_bf[6][:],
                scalar1=own_bf[6][:, i : i + 1],
                scalar2=None,
                op0=Alu.min,
            )
            nc.tensor.matmul(
                out=l1[:], lhsT=s6r[:], rhs=mb_[:], start=False, stop=True
            )
            scratch = pex.tile([K, B], bf16, tag="ex")
            nc.scalar.activation(
                out=scratch[:],
                in_=l1[:],
                func=Act.Exp,
                bias=negS[:, i : i + 1],
                scale=1.0,
                accum_out=feat[:, i : i + 1],
            )

        nc.sync.dma_start(out=feat_d[:, :], in_=feat[:])


def _build():
    if "nc" in _CACHE:
        return _CACHE["nc"]
    nc = bacc.Bacc("TRN2", target_bir_lowering=False, debug=False, num_devices=NCORES)
    x_d = nc.dram_tensor("x", [B, F], mybir.dt.float32, kind="ExternalInput").ap()
    w_d = nc.dram_tensor("w", [F, KD], mybir.dt.float32, kind="ExternalInput").ap()
    xb_d = nc.dram_tensor("xb", [ROWS, F], mybir.dt.float32, kind="ExternalInput").ap()
    feat_d = nc.dram_tensor(
        "feat", [K, ROWS], mybir.dt.float32, kind="ExternalOutput"
    ).ap()
    with tile.TileContext(nc) as tc:
        _emit(nc, tc, x_d, w_d, xb_d, feat_d)
    nc.compile()
    _CACHE["nc"] = nc
    return nc


def kernel(x, W):
    x = np.ascontiguousarray(np.asarray(x, dtype=np.float32))
    W = np.ascontiguousarray(np.asarray(W, dtype=np.float32))
    assert x.shape == (B, F) and W.shape == (F, KD)

    nc = _build()
    in_maps = [
        {"x": x, "w": W, "xb": np.ascontiguousarray(x[c * ROWS : (c + 1) * ROWS, :])}
        for c in range(NCORES)
    ]
    res = run_bass_kernel_spmd(nc, in_maps, core_ids=list(range(NCORES)))

    out = np.empty((B, F + K), dtype=np.float32)
    out[:, :F] = x
    for c in range(NCORES):
        out[c * ROWS : (c + 1) * ROWS, F:] = res.results[c]["feat"].T
    return out


# revision 17
# speedup vs baseline: 1.3865x; 1.2544x over previous
"""MiniBatchDiscrimination Trainium2 kernel (symmetric, 8-core SPMD).

out = concat([x, features], 1) where
  act = (x @ W).reshape(B, K, D)
  l1[b, b2, k] = sum_d |act[b,k,d] - act[b2,k,d]|
  features[b, k] = sum_b2 exp(-l1[b, b2, k])

Sharding: rows b are data-parallel across 8 cores (64 each). The pairwise
matrix is symmetric, so each core only computes its 64 rows against a 320-
column window: its own 64-column block plus the next 4 blocks of 64 (in
per-core "rolled" coordinates where the core's own rows sit at columns
0-63; the roll is applied on device by a permutation matmul whose operand
P is a per-core input). Pair blocks at distance 1-3 are computed once and
their mirrored contribution is exported as column sums; blocks at distance
0 and 4 are computed by both endpoint cores via row sums only. The host
adds row-sum and column-sum pieces while unsharding.

Math per tile: |a - s| = a + s - 2*min(a, s), so
  l1[k, b2] = A2[k, b2] + S[k, i] - 2*sum_d min(a, s)
with A2 = blockdiag-ones @ act (i-independent) and S[., i] = A2[., i]
(own rows are columns 0-63). A PSUM group per row i accumulates
  P = -A2 + 2*sum_d min   (the -A2 init rides a merged f32r matmul)
and ACT computes exp(P - S) with the -S column as per-partition bias,
accumulating the row sum in one pass. Column sums for blocks 1-3
accumulate over the 64 rows in a dedicated PSUM bank via an identity
matmul of the exp tile.
"""

import sys

import numpy as np

if "/opt/trn_rl_repo" not in sys.path:
    sys.path.insert(0, "/opt/trn_rl_repo")

import concourse.bass as bass  # noqa: E402
import concourse.tile as tile  # noqa: E402
from concourse import bacc, mybir  # noqa: E402
from concourse.bass_utils import run_bass_kernel_spmd  # noqa: E402
from concourse.masks import make_identity  # noqa: E402

B, F = 512, 512
K, D = 50, 16
KD = K * D  # 800
NCORES = 8
ROWS = B // NCORES  # 64 owned rows per core
NB = 5  # blocks of 64 columns each core processes (own + 4)
COLS = NB * ROWS  # 320
XCOLS = (NB - 2) * ROWS  # 192 columns whose mirrored sums are exported

_CACHE: dict = {}


def _perm_mats():
    """P_c[b, b'] = 1 iff b == (b' + 64c) mod 512, so P_c^T @ x rolls the
    rows of x by 64c (own rows land first)."""
    if "P" not in _CACHE:
        eye = np.eye(B, dtype=np.float32)
        _CACHE["P"] = [
            np.ascontiguousarray(np.roll(eye, -ROWS * c, axis=1)) for c in range(NCORES)
        ]
    return _CACHE["P"]


def _emit(nc, tc, x_d, w_d, p_d, feat_d, csum_d):
    f32 = mybir.dt.float32
    f32r = mybir.dt.float32r
    bf16 = mybir.dt.bfloat16
    Alu = mybir.AluOpType
    Act = mybir.ActivationFunctionType

    from contextlib import ExitStack

    with ExitStack() as ctx:
        persist = ctx.enter_context(tc.tile_pool(name="persist", bufs=1))
        loads = ctx.enter_context(tc.tile_pool(name="loads", bufs=4))
        ppre = ctx.enter_context(tc.tile_pool(name="ppre", bufs=3, space="PSUM"))
        pl1 = ctx.enter_context(tc.tile_pool(name="pl1", bufs=4, space="PSUM"))
        pcs = ctx.enter_context(tc.tile_pool(name="pcs", bufs=1, space="PSUM"))
        pabs = ctx.enter_context(tc.tile_pool(name="pabs", bufs=14))
        pex = ctx.enter_context(tc.tile_pool(name="pex", bufs=3))

        # --- constants -------------------------------------------------
        ident = persist.tile([128, 128], f32, tag="ident")
        make_identity(nc, ident[:])
        ident_bf = persist.tile([128, 128], bf16, tag="ident_bf")
        nc.vector.tensor_copy(ident_bf[:], ident[:])

        # ones block-diag (sums groups of 16 partitions) at columns 48..55
        # of a 98-wide tile; window j = cols [48-8j, 98-8j) puts the block at
        # output partitions 8j.. while keeping PSUM base partition 0.
        bdf32 = persist.tile([128, 98], f32, tag="bdf32")
        nc.vector.memset(bdf32[:], 1.0)
        nc.gpsimd.affine_select(
            out=bdf32[:], in_=bdf32[:],
            pattern=[[-16, 98]], channel_multiplier=1, base=768,
            compare_op=Alu.is_ge, fill=0.0,
        )
        nc.gpsimd.affine_select(
            out=bdf32[:], in_=bdf32[:],
            pattern=[[16, 98]], channel_multiplier=-1, base=-753,
            compare_op=Alu.is_ge, fill=0.0,
        )
        bdbig = persist.tile([128, 98], bf16, tag="bdbig")
        nc.vector.tensor_copy(bdbig[:], bdf32[:])
        bd2 = persist.tile([128, 98], bf16, tag="bd2")
        nc.vector.tensor_scalar(
            out=bd2[:], in0=bdf32[:], scalar1=2.0, scalar2=None, op0=Alu.mult
        )

        # merged-leftover stationary (96, 50): diag(-1) rows 0-49 and 2.0
        # blocks mapping rows 64-95 to kernels 48/49 (see v1 notes: the -A2
        # psum-init must ride the PE group; ACT->PSUM init + start=False
        # accumulation is nondeterministic on HW).
        s6f = persist.tile([96, K], f32, tag="s6f")
        nc.vector.memset(s6f[:], 0.0)
        nc.gpsimd.affine_select(
            out=s6f[:], in_=s6f[:],
            pattern=[[-1, K]], channel_multiplier=1, base=0,
            compare_op=Alu.not_equal, fill=-1.0,
        )
        aux = persist.tile([96, K], f32, tag="aux")
        nc.vector.memset(aux[:], 2.0)
        nc.gpsimd.affine_select(
            out=aux[:], in_=aux[:],
            pattern=[[-16, K]], channel_multiplier=1, base=704,
            compare_op=Alu.is_ge, fill=0.0,
        )
        nc.gpsimd.affine_select(
            out=aux[:], in_=aux[:],
            pattern=[[16, K]], channel_multiplier=-1, base=-689,
            compare_op=Alu.is_ge, fill=0.0,
        )
        nc.gpsimd.affine_select(
            out=aux[:], in_=aux[:],
            pattern=[[0, K]], channel_multiplier=1, base=-64,
            compare_op=Alu.is_ge, fill=0.0,
        )
        nc.vector.tensor_tensor(s6f[:], s6f[:], aux[:], op=Alu.add)
        s6r = persist.tile([96, K], f32r, tag="s6r")
        nc.vector.tensor_copy(s6r[:], s6f[:])

        # --- load & cast W ---------------------------------------------
        w_bf = []
        for i in range(4):
            wt = loads.tile([128, KD], f32, tag="wt")
            nc.sync.dma_start(out=wt[:], in_=w_d[128 * i : 128 * (i + 1), :])
            wb = persist.tile([128, KD], bf16, tag=f"wbf{i}")
            nc.vector.tensor_copy(wb[:], wt[:])
            w_bf.append(wb)

        # --- load x and P, cast bf16 -----------------------------------
        x_bf = []
        p_bf = []
        for i in range(4):
            xt = loads.tile([128, F], f32, tag=f"xt{i}")
            nc.sync.dma_start(out=xt[:], in_=x_d[128 * i : 128 * (i + 1), :])
            xb = persist.tile([128, F], bf16, tag=f"xbf{i}")
            nc.vector.tensor_copy(xb[:], xt[:])
            x_bf.append(xb)
            pt_ = loads.tile([128, B], f32, tag=f"pt{i}")
            nc.sync.dma_start(out=pt_[:], in_=p_d[128 * i : 128 * (i + 1), :])
            pb = persist.tile([128, B], bf16, tag=f"pbf{i}")
            nc.vector.tensor_copy(pb[:], pt_[:])
            p_bf.append(pb)

        # --- roll rows: xr = P^T @ x  (contract over b) -----------------
        xr_bf = []
        for jb in range(4):
            pr = ppre.tile([128, F], f32, tag="pp")
            for ib in range(4):
                nc.tensor.matmul(
                    out=pr[:],
                    lhsT=p_bf[ib][:, 128 * jb : 128 * (jb + 1)],
                    rhs=x_bf[ib][:],
                    start=(ib == 0),
                    stop=(ib == 3),
                )
            t = persist.tile([128, F], bf16, tag=f"xrbf{jb}")
            nc.scalar.copy(t[:], pr[:])
            xr_bf.append(t)

        # --- transpose rolled x on PE ----------------------------------
        xT_bf = []
        for fj in range(4):
            pt_ = ppre.tile([128, B], bf16, tag="pp")
            for jb in range(4):
                nc.tensor.transpose(
                    out=pt_[:, 128 * jb : 128 * (jb + 1)],
                    in_=xr_bf[jb][:, 128 * fj : 128 * (fj + 1)],
                    identity=ident_bf[:],
                )
            t = persist.tile([128, B], bf16, tag=f"xTbf{fj}")
            nc.scalar.copy(t[:], pt_[:])
            xT_bf.append(t)

        # --- act_T for the 320-column window ---------------------------
        FBLK = [(j * 128, min(128, KD - j * 128)) for j in range((KD + 127) // 128)]
        act_bf = []
        own_f32 = []
        for j, (f0, fp) in enumerate(FBLK):
            pj = ppre.tile([fp, COLS], f32, tag="pp")
            for i in range(4):
                nc.tensor.matmul(
                    out=pj[:],
                    lhsT=w_bf[i][:, f0 : f0 + fp],
                    rhs=xT_bf[i][:, 0:COLS],
                    start=(i == 0),
                    stop=(i == 3),
                )
            ab = persist.tile([fp, COLS], bf16, tag=f"actbf{j}")
            nc.scalar.copy(ab[:], pj[:])
            act_bf.append(ab)
            # own columns (0-63) as f32 scalars for the per-row min ops;
            # exactly the bf16 values so the self-term is exactly 0
            of = persist.tile([fp, ROWS], f32, tag=f"ownf{j}")
            nc.vector.tensor_copy(of[:], ab[:, 0:ROWS])
            own_f32.append(of)

        # --- A2[k, b2] = sum_{d in k} act_bf; negS = -A2[:, own] ---------
        pa2 = ppre.tile([K, COLS], f32, tag="pp")
        for j, (f0, fp) in enumerate(FBLK):
            nc.tensor.matmul(
                out=pa2[:],
                lhsT=bdbig[0:fp, 48 - 8 * j : 98 - 8 * j],
                rhs=act_bf[j][:],
                start=(j == 0),
                stop=(j == len(FBLK) - 1),
            )
        negS = persist.tile([K, ROWS], f32, tag="negS")
        nc.vector.tensor_scalar(
            out=negS[:], in0=pa2[:, 0:ROWS], scalar1=-1.0, scalar2=None, op0=Alu.mult
        )

        # triple-buffered merged moving tiles (A2 rows + leftover mins);
        # rows 50-63 face zero weights but must not hold NaN bits
        zf = loads.tile([32, COLS], f32, tag="zf")
        nc.vector.memset(zf[:], 0.0)
        m6 = []
        for b in range(3):
            t = persist.tile([96, COLS], f32r, tag=f"m6_{b}")
            nc.vector.tensor_copy(t[32:64, :], zf[:])
            nc.vector.tensor_copy(t[0:K, :], pa2[:])
            m6.append(t)

        feat = persist.tile([K, ROWS], f32, tag="feat")
        cs = pcs.tile([K, XCOLS], f32, tag="cs")

        # --- main loop over owned rows ---------------------------------
        for i in range(ROWS):
            l1 = pl1.tile([K, COLS], f32, tag="l1")
            for j in range(6):
                ab = pabs.tile([128, COLS], bf16, tag="ab")
                nc.vector.tensor_scalar(
                    out=ab[:],
                    in0=act_bf[j][:],
                    scalar1=own_f32[j][:, i : i + 1],
                    scalar2=None,
                    op0=Alu.min,
                )
                nc.tensor.matmul(
                    out=l1[:],
                    lhsT=bd2[0:128, 48 - 8 * j : 98 - 8 * j],
                    rhs=ab[:],
                    start=(j == 0),
                    stop=False,
                )
            mb_ = m6[i % 3]
            nc.vector.tensor_scalar(
                out=mb_[64:96, :],
                in0=act_bf[6][:],
                scalar1=own_f32[6][:, i : i + 1],
                scalar2=None,
                op0=Alu.min,
            )
            nc.tensor.matmul(
                out=l1[:], lhsT=s6r[:], rhs=mb_[:], start=False, stop=True
            )
            ex = pex.tile([K, COLS], bf16, tag="ex")
            nc.scalar.activation(
                out=ex[:],
                in_=l1[:],
                func=Act.Exp,
                bias=negS[:, i : i + 1],
                scale=1.0,
                accum_out=feat[:, i : i + 1],
            )
            # column sums for blocks 1-3 (mirrored contribution)
            nc.tensor.matmul(
                out=cs[:],
                lhsT=ident_bf[0:K, 0:K],
                rhs=ex[:, ROWS : ROWS + XCOLS],
                start=(i == 0),
                stop=(i == ROWS - 1),
            )

        csum_sb = persist.tile([K, XCOLS], f32, tag="csum_sb")
        nc.scalar.copy(csum_sb[:], cs[:])
        nc.sync.dma_start(out=feat_d[:, :], in_=feat[:])
        nc.sync.dma_start(out=csum_d[:, :], in_=csum_sb[:])


def _build():
    if "nc" in _CACHE:
        return _CACHE["nc"]
    nc = bacc.Bacc("TRN2", target_bir_lowering=False, debug=False, num_devices=NCORES)
    x_d = nc.dram_tensor("x", [B, F], mybir.dt.float32, kind="ExternalInput").ap()
    w_d = nc.dram_tensor("w", [F, KD], mybir.dt.float32, kind="ExternalInput").ap()
    p_d = nc.dram_tensor("p", [B, B], mybir.dt.float32, kind="ExternalInput").ap()
    feat_d = nc.dram_tensor(
        "feat", [K, ROWS], mybir.dt.float32, kind="ExternalOutput"
    ).ap()
    csum_d = nc.dram_tensor(
        "csum", [K, XCOLS], mybir.dt.float32, kind="ExternalOutput"
    ).ap()
    with tile.TileContext(nc) as tc:
        _emit(nc, tc, x_d, w_d, p_d, feat_d, csum_d)
    nc.compile()
    _CACHE["nc"] = nc
    return nc


def kernel(x, W):
    x = np.ascontiguousarray(np.asarray(x, dtype=np.float32))
    W = np.ascontiguousarray(np.asarray(W, dtype=np.float32))
    assert x.shape == (B, F) and W.shape == (F, KD)

    nc = _build()
    P = _perm_mats()
    in_maps = [{"x": x, "w": W, "p": P[c]} for c in range(NCORES)]
    res = run_bass_kernel_spmd(nc, in_maps, core_ids=list(range(NCORES)))

    feats = np.zeros((B, K), dtype=np.float32)
    for c in range(NCORES):
        feats[c * ROWS : (c + 1) * ROWS, :] += res.results[c]["feat"].T
        csum = res.results[c]["csum"]  # (K, 192): rolled cols 64..256
        for d in range(1, 4):
            rows = slice(((c + d) % NCORES) * ROWS, ((c + d) % NCORES) * ROWS + ROWS)
            feats[rows, :] += csum[:, (d - 1) * ROWS : d * ROWS].T

    out = np.empty((B, F + K), dtype=np.float32)
    out[:, :F] = x
    out[:, F:] = feats
    return out


# revision 24
# speedup vs baseline: 1.4181x; 1.0228x over previous
"""MiniBatchDiscrimination Trainium2 kernel (symmetric, 8-core SPMD).

out = concat([x, features], 1) where
  act = (x @ W).reshape(B, K, D)
  l1[b, b2, k] = sum_d |act[b,k,d] - act[b2,k,d]|
  features[b, k] = sum_b2 exp(-l1[b, b2, k])

Sharding: rows b are data-parallel across 8 cores (64 each). The pairwise
matrix is symmetric, so each core only computes its 64 rows against a 320-
column window: its own 64-column block plus the next 4 blocks of 64 (in
per-core "rolled" coordinates where the core's own rows sit at columns
0-63; the roll is applied on device by a permutation matmul whose operand
P is a per-core input). Pair blocks at distance 1-3 are computed once and
their mirrored contribution is exported as column sums; blocks at distance
0 and 4 are computed by both endpoint cores via row sums only. The host
adds row-sum and column-sum pieces while unsharding.

Math per tile: |a - s| = a + s - 2*min(a, s), so
  l1[k, b2] = A2[k, b2] + S[k, i] - 2*sum_d min(a, s)
with A2 = blockdiag-ones @ act (i-independent) and S[., i] = A2[., i]
(own rows are columns 0-63). A PSUM group per row i accumulates
  P = -A2 + 2*sum_d min   (the -A2 init rides a merged f32r matmul)
and ACT computes exp(P - S) with the -S column as per-partition bias,
accumulating the row sum in one pass. Column sums for blocks 1-3
accumulate over the 64 rows in a dedicated PSUM bank via an identity
matmul of the exp tile.
"""

import sys

import numpy as np

if "/opt/trn_rl_repo" not in sys.path:
    sys.path.insert(0, "/opt/trn_rl_repo")

import concourse.bass as bass  # noqa: E402
import concourse.tile as tile  # noqa: E402
from concourse import bacc, mybir  # noqa: E402
from concourse.bass_utils import run_bass_kernel_spmd  # noqa: E402
from concourse.masks import make_identity  # noqa: E402

B, F = 512, 512
K, D = 50, 16
KD = K * D  # 800
NCORES = 8
ROWS = B // NCORES  # 64 owned rows per core
NB = 5  # blocks of 64 columns each core processes (own + 4)
COLS = NB * ROWS  # 320
XCOLS = (NB - 2) * ROWS  # 192 columns whose mirrored sums are exported

_CACHE: dict = {}


def _perm_mats():
    """P_c[b, b'] = 1 iff b == (b' + 64c) mod 512, so P_c^T @ x rolls the
    rows of x by 64c (own rows land first)."""
    if "P" not in _CACHE:
        import ml_dtypes

        eye = np.eye(B, dtype=ml_dtypes.bfloat16)
        _CACHE["P"] = [
            np.ascontiguousarray(np.roll(eye, -ROWS * c, axis=1)) for c in range(NCORES)
        ]
    return _CACHE["P"]


def _emit(nc, tc, x_d, w_d, p_d, feat_d, csum_d):
    f32 = mybir.dt.float32
    f32r = mybir.dt.float32r
    bf16 = mybir.dt.bfloat16
    Alu = mybir.AluOpType
    Act = mybir.ActivationFunctionType

    from contextlib import ExitStack

    with ExitStack() as ctx:
        persist = ctx.enter_context(tc.tile_pool(name="persist", bufs=1))
        loads = ctx.enter_context(tc.tile_pool(name="loads", bufs=4))
        ppre = ctx.enter_context(tc.tile_pool(name="ppre", bufs=3, space="PSUM"))
        pl1 = ctx.enter_context(tc.tile_pool(name="pl1", bufs=4, space="PSUM"))
        pcs = ctx.enter_context(tc.tile_pool(name="pcs", bufs=1, space="PSUM"))
        pabs = ctx.enter_context(tc.tile_pool(name="pabs", bufs=14))
        pex = ctx.enter_context(tc.tile_pool(name="pex", bufs=3))

        # --- constants -------------------------------------------------
        ident = persist.tile([128, 128], f32, tag="ident")
        make_identity(nc, ident[:])
        ident_bf = persist.tile([128, 128], bf16, tag="ident_bf")
        nc.vector.tensor_copy(ident_bf[:], ident[:])

        # ones block-diag (sums groups of 16 partitions) at columns 48..55
        # of a 98-wide tile; window j = cols [48-8j, 98-8j) puts the block at
        # output partitions 8j.. while keeping PSUM base partition 0.
        bdf32 = persist.tile([128, 98], f32, tag="bdf32")
        nc.vector.memset(bdf32[:], 1.0)
        nc.gpsimd.affine_select(
            out=bdf32[:], in_=bdf32[:],
            pattern=[[-16, 98]], channel_multiplier=1, base=768,
            compare_op=Alu.is_ge, fill=0.0,
        )
        nc.gpsimd.affine_select(
            out=bdf32[:], in_=bdf32[:],
            pattern=[[16, 98]], channel_multiplier=-1, base=-753,
            compare_op=Alu.is_ge, fill=0.0,
        )
        bdbig = persist.tile([128, 98], bf16, tag="bdbig")
        nc.vector.tensor_copy(bdbig[:], bdf32[:])
        bd2 = persist.tile([128, 98], bf16, tag="bd2")
        nc.vector.tensor_scalar(
            out=bd2[:], in0=bdf32[:], scalar1=2.0, scalar2=None, op0=Alu.mult
        )
        bdneg = persist.tile([128, 98], bf16, tag="bdneg")
        nc.vector.tensor_scalar(
            out=bdneg[:], in0=bdf32[:], scalar1=-1.0, scalar2=None, op0=Alu.mult
        )

        # merged-leftover stationary (96, 50): diag(-1) rows 0-49 and 2.0
        # blocks mapping rows 64-95 to kernels 48/49 (see v1 notes: the -A2
        # psum-init must ride the PE group; ACT->PSUM init + start=False
        # accumulation is nondeterministic on HW).
        s6f = persist.tile([96, K], f32, tag="s6f")
        nc.vector.memset(s6f[:], 0.0)
        nc.gpsimd.affine_select(
            out=s6f[:], in_=s6f[:],
            pattern=[[-1, K]], channel_multiplier=1, base=0,
            compare_op=Alu.not_equal, fill=-1.0,
        )
        aux = persist.tile([96, K], f32, tag="aux")
        nc.vector.memset(aux[:], 2.0)
        nc.gpsimd.affine_select(
            out=aux[:], in_=aux[:],
            pattern=[[-16, K]], channel_multiplier=1, base=704,
            compare_op=Alu.is_ge, fill=0.0,
        )
        nc.gpsimd.affine_select(
            out=aux[:], in_=aux[:],
            pattern=[[16, K]], channel_multiplier=-1, base=-689,
            compare_op=Alu.is_ge, fill=0.0,
        )
        nc.gpsimd.affine_select(
            out=aux[:], in_=aux[:],
            pattern=[[0, K]], channel_multiplier=1, base=-64,
            compare_op=Alu.is_ge, fill=0.0,
        )
        nc.vector.tensor_tensor(s6f[:], s6f[:], aux[:], op=Alu.add)
        s6r = persist.tile([96, K], f32r, tag="s6r")
        nc.vector.tensor_copy(s6r[:], s6f[:])

        # --- load & cast W ---------------------------------------------
        w_bf = []
        for i in range(4):
            wt = loads.tile([128, KD], f32, tag="wt")
            nc.sync.dma_start(out=wt[:], in_=w_d[128 * i : 128 * (i + 1), :])
            wb = persist.tile([128, KD], bf16, tag=f"wbf{i}")
            nc.vector.tensor_copy(wb[:], wt[:])
            w_bf.append(wb)

        # --- load x and P, cast bf16 -----------------------------------
        x_bf = []
        p_bf = []
        for i in range(4):
            xt = loads.tile([128, F], f32, tag=f"xt{i}")
            nc.sync.dma_start(out=xt[:], in_=x_d[128 * i : 128 * (i + 1), :])
            xb = persist.tile([128, F], bf16, tag=f"xbf{i}")
            nc.vector.tensor_copy(xb[:], xt[:])
            x_bf.append(xb)
            pb = persist.tile([128, B], bf16, tag=f"pbf{i}")
            nc.sync.dma_start(out=pb[:], in_=p_d[128 * i : 128 * (i + 1), :])
            p_bf.append(pb)

        # --- roll rows: xr = P^T @ x  (contract over b) -----------------
        xr_bf = []
        for jb in range(4):
            pr = ppre.tile([128, F], f32, tag="pp")
            for ib in range(4):
                nc.tensor.matmul(
                    out=pr[:],
                    lhsT=p_bf[ib][:, 128 * jb : 128 * (jb + 1)],
                    rhs=x_bf[ib][:],
                    start=(ib == 0),
                    stop=(ib == 3),
                )
            t = persist.tile([128, F], bf16, tag=f"xrbf{jb}")
            nc.scalar.copy(t[:], pr[:])
            xr_bf.append(t)

        # --- transpose rolled x on PE ----------------------------------
        xT_bf = []
        for fj in range(4):
            pt_ = ppre.tile([128, B], bf16, tag="pp")
            for jb in range(4):
                nc.tensor.transpose(
                    out=pt_[:, 128 * jb : 128 * (jb + 1)],
                    in_=xr_bf[jb][:, 128 * fj : 128 * (fj + 1)],
                    identity=ident_bf[:],
                )
            t = persist.tile([128, B], bf16, tag=f"xTbf{fj}")
            nc.scalar.copy(t[:], pt_[:])
            xT_bf.append(t)

        # --- act_T for the 320-column window ---------------------------
        FBLK = [(j * 128, min(128, KD - j * 128)) for j in range((KD + 127) // 128)]
        act_bf = []
        own_f32 = []
        for j, (f0, fp) in enumerate(FBLK):
            pj = ppre.tile([fp, COLS], f32, tag="pp")
            for i in range(4):
                nc.tensor.matmul(
                    out=pj[:],
                    lhsT=w_bf[i][:, f0 : f0 + fp],
                    rhs=xT_bf[i][:, 0:COLS],
                    start=(i == 0),
                    stop=(i == 3),
                )
            ab = persist.tile([fp, COLS], bf16, tag=f"actbf{j}")
            nc.scalar.copy(ab[:], pj[:])
            act_bf.append(ab)
            # own columns (0-63) as f32 scalars for the per-row min ops;
            # exactly the bf16 values so the self-term is exactly 0
            of = persist.tile([fp, ROWS], f32, tag=f"ownf{j}")
            nc.vector.tensor_copy(of[:], ab[:, 0:ROWS])
            own_f32.append(of)

        # --- A2[k, b2] = sum_{d in k} act_bf; negS = -A2[:, own] ---------
        # Block 5 is handled by ACT as a direct |a-s| (ones stationary, no
        # A2/S correction), so A2/S cover only the min-route blocks.
        ACT_J = 5
        a2_blocks = [j for j in range(len(FBLK)) if j != ACT_J]
        pa2 = ppre.tile([K, COLS], f32, tag="pp")
        for n, j in enumerate(a2_blocks):
            f0, fp = FBLK[j]
            nc.tensor.matmul(
                out=pa2[:],
                lhsT=bdbig[0:fp, 48 - 8 * j : 98 - 8 * j],
                rhs=act_bf[j][:],
                start=(n == 0),
                stop=(n == len(a2_blocks) - 1),
            )
        negS = persist.tile([K, ROWS], f32, tag="negS")
        nc.vector.tensor_scalar(
            out=negS[:], in0=pa2[:, 0:ROWS], scalar1=-1.0, scalar2=None, op0=Alu.mult
        )

        # triple-buffered merged moving tiles (A2 rows + leftover mins);
        # rows 50-63 face zero weights but must not hold NaN bits
        zf = loads.tile([32, COLS], f32, tag="zf")
        nc.vector.memset(zf[:], 0.0)
        m6 = []
        for b in range(3):
            t = persist.tile([96, COLS], f32r, tag=f"m6_{b}")
            nc.vector.tensor_copy(t[32:64, :], zf[:])
            nc.vector.tensor_copy(t[0:K, :], pa2[:])
            m6.append(t)

        feat = persist.tile([K, ROWS], f32, tag="feat")
        cs = pcs.tile([K, XCOLS], f32, tag="cs")

        # --- main loop over owned rows ---------------------------------
        for i in range(ROWS):
            l1 = pl1.tile([K, COLS], f32, tag="l1")
            # ACT computes |a - s| for block 5 directly: Abs(-act + own_col)
            ab5 = pabs.tile([128, COLS], bf16, tag="ab5")
            nc.scalar.activation(
                out=ab5[:],
                in_=act_bf[ACT_J][:],
                func=Act.Abs,
                bias=own_f32[ACT_J][:, i : i + 1],
                scale=-1.0,
            )
            for j in range(5):
                ab = pabs.tile([128, COLS], bf16, tag="ab")
                nc.vector.tensor_scalar(
                    out=ab[:],
                    in0=act_bf[j][:],
                    scalar1=own_f32[j][:, i : i + 1],
                    scalar2=None,
                    op0=Alu.min,
                )
                nc.tensor.matmul(
                    out=l1[:],
                    lhsT=bd2[0:128, 48 - 8 * j : 98 - 8 * j],
                    rhs=ab[:],
                    start=(j == 0),
                    stop=False,
                )
            nc.tensor.matmul(
                out=l1[:],
                lhsT=bdneg[0:128, 48 - 8 * ACT_J : 98 - 8 * ACT_J],
                rhs=ab5[:],
                start=False,
                stop=False,
            )
            mb_ = m6[i % 3]
            nc.vector.tensor_scalar(
                out=mb_[64:96, :],
                in0=act_bf[6][:],
                scalar1=own_f32[6][:, i : i + 1],
                scalar2=None,
                op0=Alu.min,
            )
            nc.tensor.matmul(
                out=l1[:], lhsT=s6r[:], rhs=mb_[:], start=False, stop=True
            )
            ex = pex.tile([K, COLS], bf16, tag="ex")
            nc.scalar.activation(
                out=ex[:],
                in_=l1[:],
                func=Act.Exp,
                bias=negS[:, i : i + 1],
                scale=1.0,
                accum_out=feat[:, i : i + 1],
            )
            # column sums for blocks 1-3 (mirrored contribution)
            nc.tensor.matmul(
                out=cs[:],
                lhsT=ident_bf[0:K, 0:K],
                rhs=ex[:, ROWS : ROWS + XCOLS],
                start=(i == 0),
                stop=(i == ROWS - 1),
            )

        csum_sb = persist.tile([K, XCOLS], f32, tag="csum_sb")
        nc.scalar.copy(csum_sb[:], cs[:])
        nc.sync.dma_start(out=feat_d[:, :], in_=feat[:])
        nc.sync.dma_start(out=csum_d[:, :], in_=csum_sb[:])


def _build():
    if "nc" in _CACHE:
        return _CACHE["nc"]
    nc = bacc.Bacc("TRN2", target_bir_lowering=False, debug=False, num_devices=NCORES)
    x_d = nc.dram_tensor("x", [B, F], mybir.dt.float32, kind="ExternalInput").ap()
    w_d = nc.dram_tensor("w", [F, KD], mybir.dt.float32, kind="ExternalInput").ap()
    p_d = nc.dram_tensor("p", [B, B], mybir.dt.bfloat16, kind="ExternalInput").ap()
    feat_d = nc.dram_tensor(
        "feat", [K, ROWS], mybir.dt.float32, kind="ExternalOutput"
    ).ap()
    csum_d = nc.dram_tensor(
        "csum", [K, XCOLS], mybir.dt.float32, kind="ExternalOutput"
    ).ap()
    with tile.TileContext(nc) as tc:
        _emit(nc, tc, x_d, w_d, p_d, feat_d, csum_d)
    nc.compile()
    _CACHE["nc"] = nc
    return nc


def kernel(x, W):
    x = np.ascontiguousarray(np.asarray(x, dtype=np.float32))
    W = np.ascontiguousarray(np.asarray(W, dtype=np.float32))
    assert x.shape == (B, F) and W.shape == (F, KD)

    nc = _build()
    P = _perm_mats()
    in_maps = [{"x": x, "w": W, "p": P[c]} for c in range(NCORES)]
    res = run_bass_kernel_spmd(nc, in_maps, core_ids=list(range(NCORES)))

    feats = np.zeros((B, K), dtype=np.float32)
    for c in range(NCORES):
        feats[c * ROWS : (c + 1) * ROWS, :] += res.results[c]["feat"].T
        csum = res.results[c]["csum"]  # (K, 192): rolled cols 64..256
        for d in range(1, 4):
            rows = slice(((c + d) % NCORES) * ROWS, ((c + d) % NCORES) * ROWS + ROWS)
            feats[rows, :] += csum[:, (d - 1) * ROWS : d * ROWS].T

    out = np.empty((B, F + K), dtype=np.float32)
    out[:, :F] = x
    out[:, F:] = feats
    return out


# revision 27
# speedup vs baseline: 1.4494x; 1.0221x over previous
"""MiniBatchDiscrimination Trainium2 kernel (symmetric, 8-core SPMD).

out = concat([x, features], 1) where
  act = (x @ W).reshape(B, K, D)
  l1[b, b2, k] = sum_d |act[b,k,d] - act[b2,k,d]|
  features[b, k] = sum_b2 exp(-l1[b, b2, k])

Sharding: rows b are data-parallel across 8 cores (64 each). The pairwise
matrix is symmetric, so each core only computes its 64 rows against a 320-
column window: its own 64-column block plus the next 4 blocks of 64 (in
per-core "rolled" coordinates where the core's own rows sit at columns
0-63; the roll is applied on device by a permutation matmul whose operand
P is a per-core input). Pair blocks at distance 1-3 are computed once and
their mirrored contribution is exported as column sums; blocks at distance
0 and 4 are computed by both endpoint cores via row sums only. The host
adds row-sum and column-sum pieces while unsharding.

Math per tile: |a - s| = a + s - 2*min(a, s), so
  l1[k, b2] = A2[k, b2] + S[k, i] - 2*sum_d min(a, s)
with A2 = blockdiag-ones @ act (i-independent) and S[., i] = A2[., i]
(own rows are columns 0-63). A PSUM group per row i accumulates
  P = -A2 + 2*sum_d min   (the -A2 init rides a merged f32r matmul)
and ACT computes exp(P - S) with the -S column as per-partition bias,
accumulating the row sum in one pass. Column sums for blocks 1-3
accumulate over the 64 rows in a dedicated PSUM bank via an identity
matmul of the exp tile.
"""

import sys

import numpy as np

if "/opt/trn_rl_repo" not in sys.path:
    sys.path.insert(0, "/opt/trn_rl_repo")

import concourse.bass as bass  # noqa: E402
import concourse.tile as tile  # noqa: E402
from concourse import bacc, mybir  # noqa: E402
from concourse.bass_utils import run_bass_kernel_spmd  # noqa: E402
from concourse.masks import make_identity  # noqa: E402

B, F = 512, 512
K, D = 50, 16
KD = K * D  # 800
NCORES = 8
ROWS = B // NCORES  # 64 owned rows per core
NB = 5  # blocks of 64 columns each core processes (own + 4)
COLS = NB * ROWS  # 320
XCOLS = (NB - 2) * ROWS  # 192 columns whose mirrored sums are exported

_CACHE: dict = {}


def _perm_mats():
    """P_c[b, b'] = 1 iff b == (b' + 64c) mod 512, so P_c^T @ x rolls the
    rows of x by 64c (own rows land first)."""
    if "P" not in _CACHE:
        import ml_dtypes

        eye = np.eye(B, dtype=ml_dtypes.bfloat16)
        _CACHE["P"] = [
            np.ascontiguousarray(np.roll(eye, -ROWS * c, axis=1)[:, :COLS])
            for c in range(NCORES)
        ]
    return _CACHE["P"]


def _emit(nc, tc, x_d, w_d, p_d, feat_d, csum_d):
    f32 = mybir.dt.float32
    f32r = mybir.dt.float32r
    bf16 = mybir.dt.bfloat16
    Alu = mybir.AluOpType
    Act = mybir.ActivationFunctionType

    from contextlib import ExitStack

    with ExitStack() as ctx:
        persist = ctx.enter_context(tc.tile_pool(name="persist", bufs=1))
        loads = ctx.enter_context(tc.tile_pool(name="loads", bufs=4))
        ppre = ctx.enter_context(tc.tile_pool(name="ppre", bufs=3, space="PSUM"))
        pl1 = ctx.enter_context(tc.tile_pool(name="pl1", bufs=4, space="PSUM"))
        pcs = ctx.enter_context(tc.tile_pool(name="pcs", bufs=1, space="PSUM"))
        pabs = ctx.enter_context(tc.tile_pool(name="pabs", bufs=14))
        pex = ctx.enter_context(tc.tile_pool(name="pex", bufs=3))

        # --- constants -------------------------------------------------
        ident = persist.tile([128, 128], f32, tag="ident")
        make_identity(nc, ident[:])
        ident_bf = persist.tile([128, 128], bf16, tag="ident_bf")
        nc.vector.tensor_copy(ident_bf[:], ident[:])

        # ones block-diag (sums groups of 16 partitions) at columns 48..55
        # of a 98-wide tile; window j = cols [48-8j, 98-8j) puts the block at
        # output partitions 8j.. while keeping PSUM base partition 0.
        bdf32 = persist.tile([128, 98], f32, tag="bdf32")
        nc.vector.memset(bdf32[:], 1.0)
        nc.gpsimd.affine_select(
            out=bdf32[:], in_=bdf32[:],
            pattern=[[-16, 98]], channel_multiplier=1, base=768,
            compare_op=Alu.is_ge, fill=0.0,
        )
        nc.gpsimd.affine_select(
            out=bdf32[:], in_=bdf32[:],
            pattern=[[16, 98]], channel_multiplier=-1, base=-753,
            compare_op=Alu.is_ge, fill=0.0,
        )
        bdbig = persist.tile([128, 98], bf16, tag="bdbig")
        nc.vector.tensor_copy(bdbig[:], bdf32[:])
        bd2 = persist.tile([128, 98], bf16, tag="bd2")
        nc.vector.tensor_scalar(
            out=bd2[:], in0=bdf32[:], scalar1=2.0, scalar2=None, op0=Alu.mult
        )
        bdneg = persist.tile([128, 98], bf16, tag="bdneg")
        nc.vector.tensor_scalar(
            out=bdneg[:], in0=bdf32[:], scalar1=-1.0, scalar2=None, op0=Alu.mult
        )

        # merged-leftover stationary (96, 50): diag(-1) rows 0-49 and 2.0
        # blocks mapping rows 64-95 to kernels 48/49 (see v1 notes: the -A2
        # psum-init must ride the PE group; ACT->PSUM init + start=False
        # accumulation is nondeterministic on HW).
        s6f = persist.tile([96, K], f32, tag="s6f")
        nc.vector.memset(s6f[:], 0.0)
        nc.gpsimd.affine_select(
            out=s6f[:], in_=s6f[:],
            pattern=[[-1, K]], channel_multiplier=1, base=0,
            compare_op=Alu.not_equal, fill=-1.0,
        )
        aux = persist.tile([96, K], f32, tag="aux")
        nc.vector.memset(aux[:], 2.0)
        nc.gpsimd.affine_select(
            out=aux[:], in_=aux[:],
            pattern=[[-16, K]], channel_multiplier=1, base=704,
            compare_op=Alu.is_ge, fill=0.0,
        )
        nc.gpsimd.affine_select(
            out=aux[:], in_=aux[:],
            pattern=[[16, K]], channel_multiplier=-1, base=-689,
            compare_op=Alu.is_ge, fill=0.0,
        )
        nc.gpsimd.affine_select(
            out=aux[:], in_=aux[:],
            pattern=[[0, K]], channel_multiplier=1, base=-64,
            compare_op=Alu.is_ge, fill=0.0,
        )
        nc.vector.tensor_tensor(s6f[:], s6f[:], aux[:], op=Alu.add)
        s6r = persist.tile([96, K], f32r, tag="s6r")
        nc.vector.tensor_copy(s6r[:], s6f[:])

        # --- load x and P (bf16, pre-sliced to 320 cols) ----------------
        x_bf = []
        p_bf = []
        for i in range(4):
            xt = loads.tile([128, F], f32, tag=f"xt{i}")
            nc.sync.dma_start(out=xt[:], in_=x_d[128 * i : 128 * (i + 1), :])
            xb = persist.tile([128, F], bf16, tag=f"xbf{i}")
            nc.vector.tensor_copy(xb[:], xt[:])
            x_bf.append(xb)
            pb = persist.tile([128, COLS], bf16, tag=f"pbf{i}")
            nc.sync.dma_start(out=pb[:], in_=p_d[128 * i : 128 * (i + 1), :])
            p_bf.append(pb)

        # --- load W, cast on ACT (idle in the head) ---------------------
        w_bf = []
        for i in range(4):
            wt = loads.tile([128, KD], f32, tag="wt")
            nc.sync.dma_start(out=wt[:], in_=w_d[128 * i : 128 * (i + 1), :])
            wb = persist.tile([128, KD], bf16, tag=f"wbf{i}")
            nc.scalar.copy(wb[:], wt[:])
            w_bf.append(wb)

        # --- roll rows (only the 320 needed): xr = P^T @ x --------------
        RB = [(0, 128), (128, 128), (256, 64)]
        xr_bf = []
        for r0, rp in RB:
            pr = ppre.tile([rp, F], f32, tag="pp")
            for ib in range(4):
                nc.tensor.matmul(
                    out=pr[:],
                    lhsT=p_bf[ib][:, r0 : r0 + rp],
                    rhs=x_bf[ib][:],
                    start=(ib == 0),
                    stop=(ib == 3),
                )
            t = persist.tile([rp, F], bf16, tag=f"xrbf{r0}")
            nc.scalar.copy(t[:], pr[:])
            xr_bf.append(t)

        # --- transpose rolled x on PE -----------------------------------
        xT_bf = []
        for fj in range(4):
            pt_ = ppre.tile([128, COLS], bf16, tag="pp")
            for jb, (r0, rp) in enumerate(RB):
                nc.tensor.transpose(
                    out=pt_[:, r0 : r0 + rp],
                    in_=xr_bf[jb][:, 128 * fj : 128 * (fj + 1)],
                    identity=ident_bf[0:rp, 0:rp],
                )
            t = persist.tile([128, COLS], bf16, tag=f"xTbf{fj}")
            nc.scalar.copy(t[:], pt_[:])
            xT_bf.append(t)

        # --- act_T for the 320-column window ---------------------------
        FBLK = [(j * 128, min(128, KD - j * 128)) for j in range((KD + 127) // 128)]
        act_bf = []
        own_f32 = []
        for j, (f0, fp) in enumerate(FBLK):
            pj = ppre.tile([fp, COLS], f32, tag="pp")
            for i in range(4):
                nc.tensor.matmul(
                    out=pj[:],
                    lhsT=w_bf[i][:, f0 : f0 + fp],
                    rhs=xT_bf[i][:],
                    start=(i == 0),
                    stop=(i == 3),
                )
            ab = persist.tile([fp, COLS], bf16, tag=f"actbf{j}")
            nc.scalar.copy(ab[:], pj[:])
            act_bf.append(ab)
            # own columns (0-63) as f32 scalars for the per-row min ops;
            # exactly the bf16 values so the self-term is exactly 0
            of = persist.tile([fp, ROWS], f32, tag=f"ownf{j}")
            nc.vector.tensor_copy(of[:], ab[:, 0:ROWS])
            own_f32.append(of)

        # --- A2[k, b2] = sum_{d in k} act_bf; negS = -A2[:, own] ---------
        # Block 5 is handled by ACT as a direct |a-s| (ones stationary, no
        # A2/S correction), so A2/S cover only the min-route blocks.
        ACT_J = 5
        a2_blocks = [j for j in range(len(FBLK)) if j != ACT_J]
        pa2 = ppre.tile([K, COLS], f32, tag="pp")
        for n, j in enumerate(a2_blocks):
            f0, fp = FBLK[j]
            nc.tensor.matmul(
                out=pa2[:],
                lhsT=bdbig[0:fp, 48 - 8 * j : 98 - 8 * j],
                rhs=act_bf[j][:],
                start=(n == 0),
                stop=(n == len(a2_blocks) - 1),
            )
        negS = persist.tile([K, ROWS], f32, tag="negS")
        nc.vector.tensor_scalar(
            out=negS[:], in0=pa2[:, 0:ROWS], scalar1=-1.0, scalar2=None, op0=Alu.mult
        )

        # triple-buffered merged moving tiles (A2 rows + leftover mins);
        # rows 50-63 face zero weights but must not hold NaN bits
        zf = loads.tile([32, COLS], f32, tag="zf")
        nc.vector.memset(zf[:], 0.0)
        m6 = []
        for b in range(3):
            t = persist.tile([96, COLS], f32r, tag=f"m6_{b}")
            nc.vector.tensor_copy(t[32:64, :], zf[:])
            nc.vector.tensor_copy(t[0:K, :], pa2[:])
            m6.append(t)

        feat = persist.tile([K, ROWS], f32, tag="feat")
        cs = pcs.tile([K, XCOLS], f32, tag="cs")

        # --- main loop over owned rows ---------------------------------
        for i in range(ROWS):
            l1 = pl1.tile([K, COLS], f32, tag="l1")
            # ACT computes |a - s| for block 5 directly: Abs(-act + own_col)
            ab5 = pabs.tile([128, COLS], bf16, tag="ab5")
            nc.scalar.activation(
                out=ab5[:],
                in_=act_bf[ACT_J][:],
                func=Act.Abs,
                bias=own_f32[ACT_J][:, i : i + 1],
                scale=-1.0,
            )
            for j in range(5):
                ab = pabs.tile([128, COLS], bf16, tag="ab")
                nc.vector.tensor_scalar(
                    out=ab[:],
                    in0=act_bf[j][:],
                    scalar1=own_f32[j][:, i : i + 1],
                    scalar2=None,
                    op0=Alu.min,
                )
                nc.tensor.matmul(
                    out=l1[:],
                    lhsT=bd2[0:128, 48 - 8 * j : 98 - 8 * j],
                    rhs=ab[:],
                    start=(j == 0),
                    stop=False,
                )
            nc.tensor.matmul(
                out=l1[:],
                lhsT=bdneg[0:128, 48 - 8 * ACT_J : 98 - 8 * ACT_J],
                rhs=ab5[:],
                start=False,
                stop=False,
            )
            mb_ = m6[i % 3]
            nc.vector.tensor_scalar(
                out=mb_[64:96, :],
                in0=act_bf[6][:],
                scalar1=own_f32[6][:, i : i + 1],
                scalar2=None,
                op0=Alu.min,
            )
            nc.tensor.matmul(
                out=l1[:], lhsT=s6r[:], rhs=mb_[:], start=False, stop=True
            )
            ex = pex.tile([K, COLS], bf16, tag="ex")
            nc.scalar.activation(
                out=ex[:],
                in_=l1[:],
                func=Act.Exp,
                bias=negS[:, i : i + 1],
                scale=1.0,
                accum_out=feat[:, i : i + 1],
            )
            # column sums for blocks 1-3 (mirrored contribution)
            nc.tensor.matmul(
                out=cs[:],
                lhsT=ident_bf[0:K, 0:K],
                rhs=ex[:, ROWS : ROWS + XCOLS],
                start=(i == 0),
                stop=(i == ROWS - 1),
            )

        csum_sb = persist.tile([K, XCOLS], f32, tag="csum_sb")
        nc.scalar.copy(csum_sb[:], cs[:])
        nc.sync.dma_start(out=feat_d[:, :], in_=feat[:])
        nc.sync.dma_start(out=csum_d[:, :], in_=csum_sb[:])


def _build():
    if "nc" in _CACHE:
        return _CACHE["nc"]
    nc = bacc.Bacc("TRN2", target_bir_lowering=False, debug=False, num_devices=NCORES)
    x_d = nc.dram_tensor("x", [B, F], mybir.dt.float32, kind="ExternalInput").ap()
    w_d = nc.dram_tensor("w", [F, KD], mybir.dt.float32, kind="ExternalInput").ap()
    p_d = nc.dram_tensor("p", [B, COLS], mybir.dt.bfloat16, kind="ExternalInput").ap()
    feat_d = nc.dram_tensor(
        "feat", [K, ROWS], mybir.dt.float32, kind="ExternalOutput"
    ).ap()
    csum_d = nc.dram_tensor(
        "csum", [K, XCOLS], mybir.dt.float32, kind="ExternalOutput"
    ).ap()
    with tile.TileContext(nc) as tc:
        _emit(nc, tc, x_d, w_d, p_d, feat_d, csum_d)
    nc.compile()
    _CACHE["nc"] = nc
    return nc


def kernel(x, W):
    x = np.ascontiguousarray(np.asarray(x, dtype=np.float32))
    W = np.ascontiguousarray(np.asarray(W, dtype=np.float32))
    assert x.shape == (B, F) and W.shape == (F, KD)

    nc = _build()
    P = _perm_mats()
    in_maps = [{"x": x, "w": W, "p": P[c]} for c in range(NCORES)]
    res = run_bass_kernel_spmd(nc, in_maps, core_ids=list(range(NCORES)))

    feats = np.zeros((B, K), dtype=np.float32)
    for c in range(NCORES):
        feats[c * ROWS : (c + 1) * ROWS, :] += res.results[c]["feat"].T
        csum = res.results[c]["csum"]  # (K, 192): rolled cols 64..256
        for d in range(1, 4):
            rows = slice(((c + d) % NCORES) * ROWS, ((c + d) % NCORES) * ROWS + ROWS)
            feats[rows, :] += csum[:, (d - 1) * ROWS : d * ROWS].T

    out = np.empty((B, F + K), dtype=np.float32)
    out[:, :F] = x
    out[:, F:] = feats
    return out


# revision 33
# speedup vs baseline: 1.5096x; 1.0415x over previous
"""MiniBatchDiscrimination Trainium2 kernel (symmetric, 8-core SPMD).

out = concat([x, features], 1) where
  act = (x @ W).reshape(B, K, D)
  l1[b, b2, k] = sum_d |act[b,k,d] - act[b2,k,d]|
  features[b, k] = sum_b2 exp(-l1[b, b2, k])

Sharding: rows b are data-parallel across 8 cores (64 each). The pairwise
matrix is symmetric, so each core only computes its 64 rows against a 320-
column window: its own 64-column block plus the next 4 blocks of 64 (in
per-core "rolled" coordinates where the core's own rows sit at columns
0-63; the roll is applied on device by a permutation matmul whose operand
P is a per-core input). Pair blocks at distance 1-3 are computed once and
their mirrored contribution is exported as column sums; blocks at distance
0 and 4 are computed by both endpoint cores via row sums only. The host
adds row-sum and column-sum pieces while unsharding.

Math per tile: |a - s| = a + s - 2*min(a, s), so
  l1[k, b2] = A2[k, b2] + S[k, i] - 2*sum_d min(a, s)
with A2 = blockdiag-ones @ act (i-independent) and S[., i] = A2[., i]
(own rows are columns 0-63). A PSUM group per row i accumulates
  P = -A2 + 2*sum_d min   (the -A2 init rides a merged f32r matmul)
and ACT computes exp(P - S) with the -S column as per-partition bias,
accumulating the row sum in one pass. Column sums for blocks 1-3
accumulate over the 64 rows in a dedicated PSUM bank via an identity
matmul of the exp tile.
"""

import sys

import numpy as np

if "/opt/trn_rl_repo" not in sys.path:
    sys.path.insert(0, "/opt/trn_rl_repo")

import concourse.bass as bass  # noqa: E402
import concourse.tile as tile  # noqa: E402
from concourse import bacc, mybir  # noqa: E402
from concourse.bass_utils import run_bass_kernel_spmd  # noqa: E402
from concourse.masks import make_identity  # noqa: E402

B, F = 512, 512
K, D = 50, 16
KD = K * D  # 800
NCORES = 8
ROWS = B // NCORES  # 64 owned rows per core
NB = 5  # blocks of 64 columns each core processes (own + 4)
COLS = NB * ROWS  # 320
XCOLS = (NB - 2) * ROWS  # 192 columns whose mirrored sums are exported

_CACHE: dict = {}


def _perm_mats():
    """P_c[b, b'] = 1 iff b == (b' + 64c) mod 512, so P_c^T @ x rolls the
    rows of x by 64c (own rows land first)."""
    if "P" not in _CACHE:
        import ml_dtypes

        eye = np.eye(B, dtype=ml_dtypes.bfloat16)
        _CACHE["P"] = [
            np.ascontiguousarray(np.roll(eye, -ROWS * c, axis=1)[:, :COLS])
            for c in range(NCORES)
        ]
    return _CACHE["P"]


def _emit(nc, tc, x_d, w_d, p_d, feat_d, csum_d):
    f32 = mybir.dt.float32
    f32r = mybir.dt.float32r
    bf16 = mybir.dt.bfloat16
    Alu = mybir.AluOpType
    Act = mybir.ActivationFunctionType

    from contextlib import ExitStack

    with ExitStack() as ctx:
        persist = ctx.enter_context(tc.tile_pool(name="persist", bufs=1))
        loads = ctx.enter_context(tc.tile_pool(name="loads", bufs=4))
        ppre = ctx.enter_context(tc.tile_pool(name="ppre", bufs=3, space="PSUM"))
        pl1 = ctx.enter_context(tc.tile_pool(name="pl1", bufs=4, space="PSUM"))
        pcs = ctx.enter_context(tc.tile_pool(name="pcs", bufs=1, space="PSUM"))
        pabs = ctx.enter_context(tc.tile_pool(name="pabs", bufs=14))
        pex = ctx.enter_context(tc.tile_pool(name="pex", bufs=3))

        # --- constants -------------------------------------------------
        ident = persist.tile([128, 128], f32, tag="ident")
        make_identity(nc, ident[:])
        ident_bf = persist.tile([128, 128], bf16, tag="ident_bf")
        nc.vector.tensor_copy(ident_bf[:], ident[:])

        # ones block-diag (sums groups of 16 partitions) at columns 48..55
        # of a 98-wide tile; window j = cols [48-8j, 98-8j) puts the block at
        # output partitions 8j.. while keeping PSUM base partition 0.
        bdf32 = persist.tile([128, 98], f32, tag="bdf32")
        nc.vector.memset(bdf32[:], 1.0)
        nc.gpsimd.affine_select(
            out=bdf32[:], in_=bdf32[:],
            pattern=[[-16, 98]], channel_multiplier=1, base=768,
            compare_op=Alu.is_ge, fill=0.0,
        )
        nc.gpsimd.affine_select(
            out=bdf32[:], in_=bdf32[:],
            pattern=[[16, 98]], channel_multiplier=-1, base=-753,
            compare_op=Alu.is_ge, fill=0.0,
        )
        bdbig = persist.tile([128, 98], bf16, tag="bdbig")
        nc.vector.tensor_copy(bdbig[:], bdf32[:])
        bd2 = persist.tile([128, 98], bf16, tag="bd2")
        nc.vector.tensor_scalar(
            out=bd2[:], in0=bdf32[:], scalar1=2.0, scalar2=None, op0=Alu.mult
        )
        bdneg = persist.tile([128, 98], bf16, tag="bdneg")
        nc.vector.tensor_scalar(
            out=bdneg[:], in0=bdf32[:], scalar1=-1.0, scalar2=None, op0=Alu.mult
        )

        # merged-leftover stationary (96, 50): diag(-1) rows 0-49 and 2.0
        # blocks mapping rows 64-95 to kernels 48/49 (see v1 notes: the -A2
        # psum-init must ride the PE group; ACT->PSUM init + start=False
        # accumulation is nondeterministic on HW).
        s6f = persist.tile([96, K], f32, tag="s6f")
        nc.vector.memset(s6f[:], 0.0)
        nc.gpsimd.affine_select(
            out=s6f[:], in_=s6f[:],
            pattern=[[-1, K]], channel_multiplier=1, base=0,
            compare_op=Alu.not_equal, fill=-1.0,
        )
        aux = persist.tile([96, K], f32, tag="aux")
        nc.vector.memset(aux[:], 2.0)
        nc.gpsimd.affine_select(
            out=aux[:], in_=aux[:],
            pattern=[[-16, K]], channel_multiplier=1, base=704,
            compare_op=Alu.is_ge, fill=0.0,
        )
        nc.gpsimd.affine_select(
            out=aux[:], in_=aux[:],
            pattern=[[16, K]], channel_multiplier=-1, base=-689,
            compare_op=Alu.is_ge, fill=0.0,
        )
        nc.gpsimd.affine_select(
            out=aux[:], in_=aux[:],
            pattern=[[0, K]], channel_multiplier=1, base=-64,
            compare_op=Alu.is_ge, fill=0.0,
        )
        nc.vector.tensor_tensor(s6f[:], s6f[:], aux[:], op=Alu.add)
        s6r = persist.tile([96, K], f32r, tag="s6r")
        nc.vector.tensor_copy(s6r[:], s6f[:])

        # --- load x and P (bf16, pre-sliced to 320 cols) ----------------
        x_bf = []
        p_bf = []
        for i in range(4):
            xt = loads.tile([128, F], f32, tag=f"xt{i}")
            nc.sync.dma_start(out=xt[:], in_=x_d[128 * i : 128 * (i + 1), :])
            xb = persist.tile([128, F], bf16, tag=f"xbf{i}")
            nc.vector.tensor_copy(xb[:], xt[:])
            x_bf.append(xb)
            pb = persist.tile([128, COLS], bf16, tag=f"pbf{i}")
            nc.sync.dma_start(out=pb[:], in_=p_d[128 * i : 128 * (i + 1), :])
            p_bf.append(pb)

        # --- load W, cast on ACT (idle in the head) ---------------------
        w_bf = []
        for i in range(4):
            wt = loads.tile([128, KD], f32, tag="wt")
            nc.sync.dma_start(out=wt[:], in_=w_d[128 * i : 128 * (i + 1), :])
            wb = persist.tile([128, KD], bf16, tag=f"wbf{i}")
            nc.scalar.copy(wb[:], wt[:])
            w_bf.append(wb)

        # --- roll rows (only the 320 needed): xr = P^T @ x --------------
        RB = [(0, 128), (128, 128), (256, 64)]
        xr_bf = []
        for r0, rp in RB:
            pr = ppre.tile([rp, F], f32, tag="pp")
            for ib in range(4):
                nc.tensor.matmul(
                    out=pr[:],
                    lhsT=p_bf[ib][:, r0 : r0 + rp],
                    rhs=x_bf[ib][:],
                    start=(ib == 0),
                    stop=(ib == 3),
                )
            t = persist.tile([rp, F], bf16, tag=f"xrbf{r0}")
            nc.vector.tensor_copy(t[:], pr[:])
            xr_bf.append(t)

        # --- transpose rolled x on PE -----------------------------------
        xT_bf = []
        for fj in range(4):
            pt_ = ppre.tile([128, COLS], bf16, tag="pp")
            for jb, (r0, rp) in enumerate(RB):
                nc.tensor.transpose(
                    out=pt_[:, r0 : r0 + rp],
                    in_=xr_bf[jb][:, 128 * fj : 128 * (fj + 1)],
                    identity=ident_bf[0:rp, 0:rp],
                )
            t = persist.tile([128, COLS], bf16, tag=f"xTbf{fj}")
            nc.vector.tensor_copy(t[:], pt_[:])
            xT_bf.append(t)

        # --- act_T for the 320-column window ---------------------------
        FBLK = [(j * 128, min(128, KD - j * 128)) for j in range((KD + 127) // 128)]
        act_bf = []
        own_f32 = []
        for j, (f0, fp) in enumerate(FBLK):
            pj = ppre.tile([fp, COLS], f32, tag="pp")
            for i in range(4):
                nc.tensor.matmul(
                    out=pj[:],
                    lhsT=w_bf[i][:, f0 : f0 + fp],
                    rhs=xT_bf[i][:],
                    start=(i == 0),
                    stop=(i == 3),
                )
            ab = persist.tile([fp, COLS], bf16, tag=f"actbf{j}")
            nc.scalar.copy(ab[:], pj[:])
            act_bf.append(ab)
            # own columns (0-63) as f32 scalars for the per-row min ops;
            # exactly the bf16 values so the self-term is exactly 0
            of = persist.tile([fp, ROWS], f32, tag=f"ownf{j}")
            nc.vector.tensor_copy(of[:], ab[:, 0:ROWS])
            own_f32.append(of)

        # --- A2[k, b2] = sum_{d in k} act_bf; negS = -A2[:, own] ---------
        # Block 5 is handled by ACT as a direct |a-s| (ones stationary, no
        # A2/S correction), so A2/S cover only the min-route blocks.
        ACT_J = 5
        a2_blocks = [j for j in range(len(FBLK)) if j != ACT_J]
        pa2 = ppre.tile([K, COLS], f32, tag="pp")
        for n, j in enumerate(a2_blocks):
            f0, fp = FBLK[j]
            nc.tensor.matmul(
                out=pa2[:],
                lhsT=bdbig[0:fp, 48 - 8 * j : 98 - 8 * j],
                rhs=act_bf[j][:],
                start=(n == 0),
                stop=(n == len(a2_blocks) - 1),
            )
        negS = persist.tile([K, ROWS], f32, tag="negS")
        nc.vector.tensor_scalar(
            out=negS[:], in0=pa2[:, 0:ROWS], scalar1=-1.0, scalar2=None, op0=Alu.mult
        )

        # triple-buffered merged moving tiles (A2 rows + leftover mins);
        # rows 50-63 face zero weights but must not hold NaN bits
        zf = loads.tile([32, COLS], f32, tag="zf")
        nc.vector.memset(zf[:], 0.0)
        m6 = []
        for b in range(3):
            t = persist.tile([96, COLS], f32r, tag=f"m6_{b}")
            nc.vector.tensor_copy(t[32:64, :], zf[:])
            nc.vector.tensor_copy(t[0:K, :], pa2[:])
            m6.append(t)

        feat = persist.tile([K, ROWS], f32, tag="feat")
        cs = pcs.tile([K, XCOLS], f32, tag="cs")

        # --- main loop over owned rows ---------------------------------
        for i in range(ROWS):
            l1 = pl1.tile([K, COLS], f32, tag="l1")
            # ACT computes |a - s| for block 5 directly: Abs(-act + own_col)
            ab5 = pabs.tile([128, COLS], bf16, tag="ab5")
            nc.scalar.activation(
                out=ab5[:],
                in_=act_bf[ACT_J][:],
                func=Act.Abs,
                bias=own_f32[ACT_J][:, i : i + 1],
                scale=-1.0,
            )
            for j in range(5):
                ab = pabs.tile([128, COLS], bf16, tag="ab")
                nc.vector.tensor_scalar(
                    out=ab[:],
                    in0=act_bf[j][:],
                    scalar1=own_f32[j][:, i : i + 1],
                    scalar2=None,
                    op0=Alu.min,
                )
                nc.tensor.matmul(
                    out=l1[:],
                    lhsT=bd2[0:128, 48 - 8 * j : 98 - 8 * j],
                    rhs=ab[:],
                    start=(j == 0),
                    stop=False,
                )
            nc.tensor.matmul(
                out=l1[:],
                lhsT=bdneg[0:128, 48 - 8 * ACT_J : 98 - 8 * ACT_J],
                rhs=ab5[:],
                start=False,
                stop=False,
            )
            mb_ = m6[i % 3]
            nc.vector.tensor_scalar(
                out=mb_[64:96, :],
                in0=act_bf[6][:],
                scalar1=own_f32[6][:, i : i + 1],
                scalar2=None,
                op0=Alu.min,
            )
            nc.tensor.matmul(
                out=l1[:], lhsT=s6r[:], rhs=mb_[:], start=False, stop=True
            )
            ex = pex.tile([K, COLS], bf16, tag="ex")
            nc.scalar.activation(
                out=ex[:],
                in_=l1[:],
                func=Act.Exp,
                bias=negS[:, i : i + 1],
                scale=1.0,
                accum_out=feat[:, i : i + 1],
            )
            # column sums for blocks 1-3 (mirrored contribution)
            nc.tensor.matmul(
                out=cs[:],
                lhsT=ident_bf[0:K, 0:K],
                rhs=ex[:, ROWS : ROWS + XCOLS],
                start=(i == 0),
                stop=(i == ROWS - 1),
            )

        csum_sb = persist.tile([K, XCOLS], f32, tag="csum_sb")
        nc.scalar.copy(csum_sb[:], cs[:])
        nc.sync.dma_start(out=feat_d[:, :], in_=feat[:])
        nc.sync.dma_start(out=csum_d[:, :], in_=csum_sb[:])


def _build():
    if "nc" in _CACHE:
        return _CACHE["nc"]
    nc = bacc.Bacc("TRN2", target_bir_lowering=False, debug=False, num_devices=NCORES)
    x_d = nc.dram_tensor("x", [B, F], mybir.dt.float32, kind="ExternalInput").ap()
    w_d = nc.dram_tensor("w", [F, KD], mybir.dt.float32, kind="ExternalInput").ap()
    p_d = nc.dram_tensor("p", [B, COLS], mybir.dt.bfloat16, kind="ExternalInput").ap()
    feat_d = nc.dram_tensor(
        "feat", [K, ROWS], mybir.dt.float32, kind="ExternalOutput"
    ).ap()
    csum_d = nc.dram_tensor(
        "csum", [K, XCOLS], mybir.dt.float32, kind="ExternalOutput"
    ).ap()
    with tile.TileContext(nc) as tc:
        _emit(nc, tc, x_d, w_d, p_d, feat_d, csum_d)
    nc.compile()
    _CACHE["nc"] = nc
    return nc


def _get_runner():
    """Build the 8-core PJRT executable once and reuse it across calls
    (run_bass_kernel_spmd re-traces and re-jits per call)."""
    if "run" in _CACHE:
        return _CACHE["run"]
    nc = _build()

    import jax
    from jax.sharding import Mesh, PartitionSpec
    try:
        from jax.experimental.shard_map import shard_map
    except ImportError:  # newer jax
        from jax.shard_map import shard_map
    from concourse import bass2jax, mybir as mb

    bass2jax.install_neuronx_cc_hook()

    in_names: list[str] = []
    out_names: list[str] = []
    out_avals = []
    zero_shapes = []
    for alloc in nc.m.functions[0].allocations:
        if not isinstance(alloc, mb.MemoryLocationSet):
            continue
        name = alloc.memorylocations[0].name
        if alloc.kind == "ExternalInput":
            if nc.partition_id_tensor and name == nc.partition_id_tensor.name:
                continue
            in_names.append(name)
        elif alloc.kind == "ExternalOutput":
            out_names.append(name)
            shape = tuple(alloc.tensor_shape)
            dtype = mb.dt.np(alloc.dtype)
            out_avals.append(jax.core.ShapedArray(shape, dtype))
            zero_shapes.append((shape, dtype))
    n_params = len(in_names)
    n_outs = len(out_names)
    all_names = in_names + out_names
    pname = nc.partition_id_tensor.name if nc.partition_id_tensor else None
    if pname is not None:
        all_names = all_names + [pname]

    def _body(*args):
        operands = list(args)
        if pname is not None:
            operands.append(bass2jax.partition_id_tensor())
        outs = bass2jax._bass_exec_p.bind(
            *operands,
            out_avals=tuple(out_avals),
            in_names=tuple(all_names),
            out_names=tuple(out_names),
            lowering_input_output_aliases=(),
            sim_require_finite=True,
            sim_require_nnan=True,
            nc=nc,
        )
        return tuple(outs)

    devices = jax.devices()[:NCORES]
    mesh = Mesh(np.asarray(devices), ("core",))
    specs = (PartitionSpec("core"),) * (n_params + n_outs)
    sharded = jax.jit(
        shard_map(
            _body,
            mesh=mesh,
            in_specs=specs,
            out_specs=(PartitionSpec("core"),) * n_outs,
            check_rep=False,
        ),
        donate_argnums=tuple(range(n_params, n_params + n_outs)),
        keep_unused=True,
    )

    def run(in_maps):
        concat_in = [
            np.concatenate([np.asarray(m[name]) for m in in_maps], axis=0)
            for name in in_names
        ]
        zeros = [
            np.zeros((NCORES * s[0], *s[1:]), dt) for s, dt in zero_shapes
        ]
        out_arrs = sharded(*concat_in, *zeros)
        return [
            {
                name: np.asarray(out_arrs[i]).reshape(
                    NCORES, *zero_shapes[i][0]
                )[c]
                for i, name in enumerate(out_names)
            }
            for c in range(NCORES)
        ]

    _CACHE["run"] = run
    return run


def kernel(x, W):
    x = np.ascontiguousarray(np.asarray(x, dtype=np.float32))
    W = np.ascontiguousarray(np.asarray(W, dtype=np.float32))
    assert x.shape == (B, F) and W.shape == (F, KD)

    run = _get_runner()
    P = _perm_mats()
    in_maps = [{"x": x, "w": W, "p": P[c]} for c in range(NCORES)]
    results = run(in_maps)

    feats = np.zeros((B, K), dtype=np.float32)
    for c in range(NCORES):
        feats[c * ROWS : (c + 1) * ROWS, :] += results[c]["feat"].T
        csum = results[c]["csum"]  # (K, 192): rolled cols 64..256
        for d in range(1, 4):
            rows = slice(((c + d) % NCORES) * ROWS, ((c + d) % NCORES) * ROWS + ROWS)
            feats[rows, :] += csum[:, (d - 1) * ROWS : d * ROWS].T

    out = np.empty((B, F + K), dtype=np.float32)
    out[:, :F] = x
    out[:, F:] = feats
    return out
